# revision 61
# baseline (speedup 1.0000x reference)
"""BLOOM attention block (B=2, S=2048, D=2048, H=16) on 8 Trainium2 NeuronCores.

Sharding: core c handles batch b=c//4 and head slot group g=c%4 with a
STRIDE-4 head assignment (slot i holds global head g+4i), so slots 2,3 only
ever see small-ALiBi-slope heads. Each core computes its 4 heads' attention
plus the partial dense projection (W_dense columns for its heads); the host
sums the 4 bf16 partials per batch and adds b_dense + residual in fp32.

Device-side layout avoids all large on-chip transposes:
  - The projection emits Q^T, K^T in [head_dim(=128 partitions), seq] layout
    and V in native [seq, head_dim] layout; the V projection bias is folded
    into V itself during psum evacuation (exact: probs sum to 1).
  - scores are computed transposed: S^T[sk, sq] = K @ Q^T.
  - softmax over sk (the partition dim) uses an analytic shift c[sq] =
    cummax(alibi) per column (any per-column shift cancels in the
    normalization). Slot 0 (big slopes) injects -c via a rank-1 K=1 matmul;
    slot 1 folds per-HALF-quarter constant shifts into two exp biases
    (alib/alib2); slots 2,3 fold a per-quarter constant into one exp bias.
    Underflow analysis: a constant shift over a span W is safe iff
    slope*W + |qk| stays below bf16's exp underflow (~87).
  - column sums come nearly free: per 128-column chunk, a matmul with the
    pt chunk STATIONARY and a ones column moving (N=1, cost-model charges
    by output free size). The sums land as per-partition columns [sq,1],
    are inverted on DVE ([128,4]), PE-transposed to [4,128], and broadcast
    across hd partitions with 4 rank-1 one-hot bf16 matmuls.
  - PSUM zero-region rule: start=True marks the whole 2KB bank pending-zero,
    so only the FIRST write of each bank occupancy carries start=True; each
    later chunk's first write overwrites-and-arms its own bytes.
  - ctx^T[hd, sq] = V^T @ P^T accumulates per 128-chunk in PSUM with
    per-chunk stop at each chunk's true last writer (causal boundary tiles).
  - dense partial OUT[sq, dout] = ctx^T.T @ W_dense^T accumulated over heads;
    dense do-chunks of the previous quarter are interleaved between the
    normalize matmuls of the current quarter's heads to keep the PE fed, and
    the tail quarter streams its stores per do-chunk.
"""

import math
import time

import numpy as np

import bass_rust
import concourse.bass as bass
import concourse.mybir as mybir
import concourse.tile as tile
from concourse import bass_utils

import ml_dtypes

BF16_NP = ml_dtypes.bfloat16

B, S, D, H = 2, 2048, 2048, 16
HD = D // H  # 128
INV_NORM = 1.0 / math.sqrt(HD)
NCORES = 8
HPC = 4  # heads per core
SQT = 512  # sq tile width (free dim of transposed score tiles)
NQT = S // SQT  # 4
NKT = S // 128  # 16 sk tiles
NDT = D // 128  # 16 contraction tiles
FD32 = mybir.dt.float32
BF16 = mybir.dt.bfloat16
NEG_BIG = -1.0e9
PSUM_QPS = 0
PSUM_QKV = 4
WORK_BUFS = 4
PSUM_SMPS = 1
PSUM_SCPS = 4
PSUM_CTXPS = 2
PSUM_DPS = 1
QJ_ORDER = [3, 2, 0, 1]
BOUNDARY_ON_POOL = True
SHARE_QD = False
EXP_SPLIT = False
QX2_BUFS = 2
PT_BUFS = 4
CTXT_BUFS = 2
OUTSB_BUFS = 3


def _split_multi_waits(nc):
    """This toolchain's walrus accepts at most ONE sync wait per instruction;
    Tile emits multi-wait instructions. Move extra waits onto preceding NOPs
    on the same engine (waits execute in stream order, so semantics hold)."""
    for fn in nc.m.functions:
        for bb in fn.blocks:
            insts = bb.instructions
            i = 0
            while i < len(insts):
                inst = insts[i]
                si = inst.sync_info
                if si is not None and len(si.on_wait) > 1:
                    waits = list(si.on_wait)
                    carriers = []
                    for k, w in enumerate(waits[:-1]):
                        nop = mybir.InstNoOp(name=f"{inst.name}_sw{k}", ins=[], outs=[])
                        nop.engine = inst.engine
                        nop.sync_info = bass_rust.SyncInfo(on_wait=[w], on_update=[])
                        nc.register_instruction(nop, overwrite=True)
                        carriers.append(nop)
                    inst.sync_info = bass_rust.SyncInfo(
                        on_wait=[waits[-1]], on_update=si.on_update
                    )
                    insts[i:i] = carriers
                    i += len(carriers)
                i += 1


def _tile_plan(mode):
    """plan[qj][ki] in {'skip','clean','pat'} ('pat' only in causal mode;
    'data' mode returns 'data' everywhere)."""
    plan = []
    for qj in range(NQT):
        row = []
        for ki in range(NKT):
            if mode == "none":
                row.append("clean")
            elif mode == "data":
                row.append("data")
            else:  # causal: keys sk <= queries sq
                sk_lo, sk_hi = 128 * ki, 128 * ki + 127
                sq_lo, sq_hi = SQT * qj, SQT * qj + SQT - 1
                if sk_lo > sq_hi:
                    row.append("skip")
                elif sk_hi <= sq_lo:
                    row.append("clean")
                else:
                    row.append("pat")  # pattern index = ki - 4*qj
        plan.append(row)
    return plan


def _build_program(mode):
    """mode in {'none', 'causal', 'data'}; returns the Bass module."""
    plan = _tile_plan(mode)
    use_shift = mode != "none"  # 'none' folds the constant shift into alib
    # In causal mode slots 2,3 hold only small-slope heads (stride-4 head
    # assignment): a per-(head, quarter) constant shift folded into the exp
    # bias stays within bf16 range, so the rank-1 shift matmul is dropped.
    # Slot 1 (slopes <= 0.177) gets the same treatment with per-HALF-quarter
    # constants (two exp activations per tile, biases from alib/alib2).
    shift_slots = (0,) if mode == "causal" else tuple(range(HPC))
    half_slots = (1,) if mode == "causal" else ()

    nc = bass.Bass()
    xt = nc.dram_tensor("xt", [D, S], BF16, kind="ExternalInput")
    wqt = nc.dram_tensor("wqt", [D, HPC * HD], BF16, kind="ExternalInput")
    wkt = nc.dram_tensor("wkt", [D, HPC * HD], BF16, kind="ExternalInput")
    wvt = nc.dram_tensor("wvt", [D, HPC * HD], BF16, kind="ExternalInput")
    wdt = nc.dram_tensor("wdt", [HPC * HD, D], BF16, kind="ExternalInput")
    bqk = nc.dram_tensor("bqk", [128, 2 * HPC], FD32, kind="ExternalInput")
    bvbc = nc.dram_tensor("bvbc", [128, HPC * HD], FD32, kind="ExternalInput")
    # exp bias per (slot-head, quarter, ki): for shift-free slots the
    # per-quarter constant shift is folded in, so columns vary with qj.
    alib = nc.dram_tensor("alib", [128, HPC * NQT * NKT], FD32, kind="ExternalInput")
    alib2 = nc.dram_tensor("alib2", [128, HPC * NQT * NKT], FD32, kind="ExternalInput")
    ones1p = nc.dram_tensor("ones1p", [1, 128], BF16, kind="ExternalInput")
    onesp1 = nc.dram_tensor("onesp1", [128, 1], BF16, kind="ExternalInput")
    ident = nc.dram_tensor("ident", [128, 128], FD32, kind="ExternalInput")
    oneh = nc.dram_tensor("oneh", [4, 4 * 128], BF16, kind="ExternalInput")
    negc = patt = maskt = None
    if use_shift:
        negc = nc.dram_tensor("negc", [1, HPC * S], BF16, kind="ExternalInput")
    if mode == "causal":
        patt = nc.dram_tensor("patt", [128, 4 * SQT], FD32, kind="ExternalInput")
    if mode == "data":
        maskt = nc.dram_tensor("maskt", [S, S], FD32, kind="ExternalInput")
    outp = nc.dram_tensor("outp", [S, D], BF16, kind="ExternalOutput")

    with tile.TileContext(nc) as tc:
        with tc.tile_pool(name="persist", bufs=1) as persist:
            # ---- persistent SBUF tensors -------------------------------
            # Small constants first (cheap DMAs, needed early).
            qt_sb = persist.tile([128, HPC, S], BF16)  # Q^T per head
            kt_sb = persist.tile([128, HPC, S], BF16)  # K^T per head
            v_sb = persist.tile([128, NKT, HPC * HD], BF16)  # V native
            wdt_sb = persist.tile([128, HPC, D], BF16)
            bqk_sb = persist.tile([128, 2 * HPC], FD32)
            bvbc_sb = persist.tile([128, HPC * HD], FD32)
            alib_sb = persist.tile([128, HPC * NQT * NKT], FD32)
            alib2_sb = persist.tile([128, HPC * NQT * NKT], FD32)
            ones1p_sb = persist.tile([1, 128], BF16)
            onesp1_sb = persist.tile([128, 1], BF16)
            ident_sb = persist.tile([128, 128], FD32)
            oneh_sb = persist.tile([4, 4 * 128], BF16)
            negc_sb = patt_sb = None
            if use_shift:
                negc_sb = persist.tile([1, HPC * S], BF16)
            if mode == "causal":
                patt_sb = persist.tile([128, 4, SQT], FD32)

            def emit_const_dmas():
                # emitted AFTER the critical first weight/xt chunks so the
                # shared descriptor-gen engine serves those first.
                nc.gpsimd.dma_start(out=bqk_sb, in_=bqk[:])
                nc.gpsimd.dma_start(out=bvbc_sb, in_=bvbc[:])
                nc.gpsimd.dma_start(out=alib_sb, in_=alib[:])
                nc.gpsimd.dma_start(out=alib2_sb, in_=alib2[:])
                nc.gpsimd.dma_start(out=ones1p_sb, in_=ones1p[:])
                nc.gpsimd.dma_start(out=onesp1_sb, in_=onesp1[:])
                nc.gpsimd.dma_start(out=ident_sb, in_=ident[:])
                nc.gpsimd.dma_start(out=oneh_sb, in_=oneh[:])
                if use_shift:
                    nc.gpsimd.dma_start(out=negc_sb, in_=negc[:])

            # ---- phase 1: K+V projection (Q is interleaved into phase 2)
            xt_r = xt.rearrange("(dt p) s -> p dt s", p=128)
            wqt_r = wqt.rearrange("(dt p) f -> p dt f", p=128)
            wkt_r = wkt.rearrange("(dt p) f -> p dt f", p=128)
            wvt_r = wvt.rearrange("(dt p) f -> p dt f", p=128)
            with tc.tile_pool(name="wqp", bufs=1) as wqp:
                wq_sb = wqp.tile([128, NDT, HPC * HD], BF16)
                with (
                    tc.tile_pool(name="qkvw", bufs=1) as qkvw,
                    tc.tile_pool(name="qkvx", bufs=2) as qkvx,
                    tc.tile_pool(name="qkvps", bufs=PSUM_QKV, space="PSUM") as qkvps,
                ):
                    # Chunked loads (4 dt-groups each) so the first matmuls
                    # can start as soon as the first chunk lands. The first
                    # wk chunk and first xt chunk go ahead of the constants.
                    wk_sb = qkvw.tile([128, NDT, HPC * HD], BF16)
                    wv_sb = qkvw.tile([128, NDT, HPC * HD], BF16)
                    nc.sync.dma_start(out=wk_sb[:, 0:1, :], in_=wkt_r[:, 0:1, :])
                    xt_q0 = qkvx.tile([128, NDT, SQT], BF16, name="xt_q")
                    nc.scalar.dma_start(out=xt_q0[:, 0:1, :], in_=xt_r[:, 0:1, 0:SQT])
                    nc.sync.dma_start(out=wk_sb[:, 1:4, :], in_=wkt_r[:, 1:4, :])
                    nc.scalar.dma_start(out=xt_q0[:, 1:4, :], in_=xt_r[:, 1:4, 0:SQT])
                    emit_const_dmas()
                    for c4 in range(1, 4):
                        dsl = slice(c4 * 4, (c4 + 1) * 4)
                        nc.sync.dma_start(out=wk_sb[:, dsl, :], in_=wkt_r[:, dsl, :])
                    for c4 in range(4):
                        dsl = slice(c4 * 4, (c4 + 1) * 4)
                        nc.sync.dma_start(out=wv_sb[:, dsl, :], in_=wvt_r[:, dsl, :])
                    for c4 in range(4):
                        dsl = slice(c4 * 4, (c4 + 1) * 4)
                        nc.sync.dma_start(out=wq_sb[:, dsl, :], in_=wqt_r[:, dsl, :])
                    for q in range(4):  # seq quarters of 512
                        sq0 = q * SQT
                        if q == 0:
                            xt_q = xt_q0
                            c4range = range(1, 4)
                        else:
                            xt_q = qkvx.tile([128, NDT, SQT], BF16, name="xt_q")
                            c4range = range(4)
                        for c4 in c4range:
                            dsl = slice(c4 * 4, (c4 + 1) * 4)
                            nc.scalar.dma_start(
                                out=xt_q[:, dsl, :], in_=xt_r[:, dsl, sq0 : sq0 + SQT]
                            )
                        if q == 3:
                            # dense weights are needed only at the first dense
                            # block (~150us in); keep them behind all xt loads.
                            for c4 in range(4):
                                nc.scalar.dma_start(
                                    out=wdt_sb[:, c4, :],
                                    in_=wdt.rearrange("(h p) o -> p h o", p=128)[
                                        :, c4, :
                                    ],
                                )
                        for h in range(HPC):
                            ps_k = qkvps.tile([128, SQT], FD32, tag="qkvps")
                            for dt in range(NDT):
                                nc.tensor.matmul(
                                    ps_k,
                                    wk_sb[:, dt, h * HD : (h + 1) * HD],
                                    xt_q[:, dt, :],
                                    start=(dt == 0),
                                    stop=(dt == NDT - 1),
                                )
                            nc.vector.tensor_scalar_add(
                                kt_sb[:, h, sq0 : sq0 + SQT],
                                ps_k,
                                bqk_sb[:, HPC + h : HPC + h + 1],
                            )
                        for sc in range(4):  # V rows within the quarter
                            ps_v = qkvps.tile([128, SQT], FD32, tag="qkvps")
                            for dt in range(NDT):
                                nc.tensor.matmul(
                                    ps_v,
                                    xt_q[:, dt, sc * 128 : (sc + 1) * 128],
                                    wv_sb[:, dt, :],
                                    start=(dt == 0),
                                    stop=(dt == NDT - 1),
                                )
                            # V carries its projection bias: exact, since the
                            # normalized probs per column sum to 1, so ctx/sum
                            # picks up + bv without a separate rank-1 fold.
                            nc.vector.tensor_tensor(
                                out=v_sb[:, q * 4 + sc, :],
                                in0=ps_v,
                                in1=bvbc_sb,
                                op=mybir.AluOpType.add,
                            )
                        if q == QJ_ORDER[0]:
                            # Q for the first attention block: computed here
                            # while its xt quarter is still resident, so
                            # attention can start the moment K/V complete.
                            for h in range(HPC):
                                ps_q = qkvps.tile([128, SQT], FD32, tag="qkvps")
                                for dt in range(NDT):
                                    nc.tensor.matmul(
                                        ps_q,
                                        wq_sb[:, dt, h * HD : (h + 1) * HD],
                                        xt_q[:, dt, :],
                                        start=(dt == 0),
                                        stop=(dt == NDT - 1),
                                    )
                                nc.vector.tensor_scalar_add(
                                    qt_sb[:, h, sq0 : sq0 + SQT],
                                    ps_q,
                                    bqk_sb[:, h : h + 1],
                                )

                # ---- phases 2+3: Q projection + attention + dense, per sq
                # block of 512; Q matmuls interleave with attention to keep
                # the PE fed across unit boundaries.
                with (
                    tc.tile_pool(name="qx2", bufs=QX2_BUFS) as qx2,
                    tc.tile_pool(name="work", bufs=WORK_BUFS) as work,
                    tc.tile_pool(name="ctxtp", bufs=CTXT_BUFS) as ctxtp,
                    tc.tile_pool(name="outsb", bufs=OUTSB_BUFS) as outsb,
                    tc.tile_pool(name="maskp", bufs=2) as maskp,
                ):

                    def emit_dense_do(
                        sq0, ctxt_sb, sc, do, pool, out_sb, tag="dps", dve_only=False
                    ):
                        o_ps = pool.tile([128, 512], FD32, tag=tag, name="o_ps")
                        for h in range(HPC):
                            nc.tensor.matmul(
                                o_ps,
                                ctxt_sb[:, h, sc * 128 : (sc + 1) * 128],
                                wdt_sb[:, h, do * 512 : (do + 1) * 512],
                                start=(h == 0),
                                stop=(h == HPC - 1),
                            )
                        if dve_only or do % 2 == 0:
                            nc.vector.tensor_copy(
                                out_sb[:, do * 512 : (do + 1) * 512], o_ps
                            )
                        else:
                            nc.scalar.copy(out_sb[:, do * 512 : (do + 1) * 512], o_ps)

                    def emit_dense(sq0, ctxt_sb, pool, tag="dps", stream_store=False):
                        for sc in range(4):
                            r0 = sq0 + sc * 128
                            if stream_store:
                                # tail: stream each do-chunk's store right
                                # after its evac copy.
                                out_sb = outsb.tile([128, D], BF16, name="out_sb")
                                for do in range(4):
                                    emit_dense_do(
                                        sq0, ctxt_sb, sc, do, pool, out_sb, tag
                                    )
                                    nc.sync.dma_start(
                                        out=outp[
                                            r0 : r0 + 128, do * 512 : (do + 1) * 512
                                        ],
                                        in_=out_sb[:, do * 512 : (do + 1) * 512],
                                    )
                            else:
                                out_sb = outsb.tile([128, D], BF16, name="out_sb")
                                for do in range(4):
                                    emit_dense_do(
                                        sq0, ctxt_sb, sc, do, pool, out_sb, tag
                                    )
                                nc.sync.dma_start(
                                    out=outp[r0 : r0 + 128, :], in_=out_sb
                                )

                    prev_dense = None
                    with (
                        tc.tile_pool(name="qps", bufs=max(PSUM_QPS, 1), space="PSUM") as qps0,
                        tc.tile_pool(
                            name="scps", bufs=PSUM_SCPS, space="PSUM"
                        ) as scps,
                        tc.tile_pool(
                            name="ctxps", bufs=PSUM_CTXPS, space="PSUM"
                        ) as ctxps,
                        tc.tile_pool(name="smps", bufs=PSUM_SMPS, space="PSUM") as smps,
                        tc.tile_pool(name="dps", bufs=PSUM_DPS, space="PSUM") as dps,
                    ):
                        qps = qps0 if PSUM_QPS > 0 else scps
                        qtag = "qps" if PSUM_QPS > 0 else "scps"
                        for qj in QJ_ORDER:
                            sq0 = qj * SQT
                            if qj != QJ_ORDER[0]:
                                xt_q = qx2.tile([128, NDT, SQT], BF16)
                                for c4 in range(4):
                                    dsl = slice(c4 * 4, (c4 + 1) * 4)
                                    nc.scalar.dma_start(
                                        out=xt_q[:, dsl, :],
                                        in_=xt_r[:, dsl, sq0 : sq0 + SQT],
                                    )
                                for h in range(HPC):
                                    ps_q = qps.tile([128, SQT], FD32, tag=qtag, name="ps_q")
                                    for dt in range(NDT):
                                        nc.tensor.matmul(
                                            ps_q,
                                            wq_sb[:, dt, h * HD : (h + 1) * HD],
                                            xt_q[:, dt, :],
                                            start=(dt == 0),
                                            stop=(dt == NDT - 1),
                                        )
                                    nc.vector.tensor_scalar_add(
                                        qt_sb[:, h, sq0 : sq0 + SQT],
                                        ps_q,
                                        bqk_sb[:, h : h + 1],
                                    )
                            ctxt_sb = ctxtp.tile([128, HPC, SQT], BF16)
                            for h in range(HPC):
                                ki_list = [
                                    ki for ki in range(NKT) if plan[qj][ki] != "skip"
                                ]
                                nlast = len(ki_list) - 1
                                ctx_ps = ctxps.tile([128, SQT], FD32, tag="ctxps")
                                sums_ps = smps.tile([128, 4], FD32, tag="smps")
                                for n, ki in enumerate(ki_list):
                                    kind = plan[qj][ki]
                                    # boundary tiles: sq columns below the
                                    # diagonal block are fully masked -- skip
                                    # them (the first tile of each unit is
                                    # always full width, so the psum
                                    # accumulation start covers all columns).
                                    off = 0
                                    if kind == "pat":
                                        off = 128 * (ki - 4 * qj)
                                    w = SQT - off
                                    q0o = sq0 + off
                                    h_shift = use_shift and h in shift_slots
                                    s_ps = scps.tile([128, SQT], FD32, tag="scps")
                                    if h_shift:
                                        nc.tensor.matmul(
                                            s_ps[:, off:SQT],
                                            ones1p_sb,
                                            negc_sb[0:1, h * S + q0o : h * S + sq0 + SQT],
                                            start=True,
                                            stop=False,
                                        )
                                    nc.tensor.matmul(
                                        s_ps[:, off:SQT],
                                        kt_sb[:, h, ki * 128 : (ki + 1) * 128],
                                        qt_sb[:, h, q0o : sq0 + SQT],
                                        start=not h_shift,
                                        stop=True,
                                    )
                                    if kind == "pat" and not BOUNDARY_ON_POOL:
                                        nc.vector.tensor_tensor(
                                            out=s_ps[:, off:SQT],
                                            in0=s_ps[:, off:SQT],
                                            in1=patt_sb[:, ki - 4 * qj, off:SQT],
                                            op=mybir.AluOpType.add,
                                        )
                                    elif kind == "data":
                                        mk_sb = maskp.tile([128, SQT], FD32, tag="mask")
                                        nc.sync.dma_start(
                                            out=mk_sb,
                                            in_=maskt[
                                                ki * 128 : (ki + 1) * 128, sq0 : sq0 + SQT
                                            ],
                                        )
                                        nc.vector.tensor_tensor(
                                            out=s_ps,
                                            in0=s_ps,
                                            in1=mk_sb,
                                            op=mybir.AluOpType.add,
                                        )
                                    pt_sb = work.tile([128, SQT], BF16, tag="pt", bufs=PT_BUFS)
                                    bcol = (h * NQT + qj) * NKT + ki
                                    if h in half_slots:
                                        # per-half-quarter constant shifts:
                                        # first half bias from alib, second
                                        # from alib2 (both fold their own c).
                                        if off < 256:
                                            nc.scalar.activation(
                                                pt_sb[:, 0 : 256 - off],
                                                s_ps[:, off:256],
                                                mybir.ActivationFunctionType.Exp,
                                                bias=alib_sb[:, bcol : bcol + 1],
                                            )
                                        lo = max(off, 256)
                                        nc.scalar.activation(
                                            pt_sb[:, lo - off : SQT - off],
                                            s_ps[:, lo:SQT],
                                            mybir.ActivationFunctionType.Exp,
                                            bias=alib2_sb[:, bcol : bcol + 1],
                                        )
                                    else:
                                        nc.scalar.activation(
                                            pt_sb[:, 0:w],
                                            s_ps[:, off:SQT],
                                            mybir.ActivationFunctionType.Exp,
                                            bias=alib_sb[:, bcol : bcol + 1],
                                        )
                                    if kind == "pat" and BOUNDARY_ON_POOL:
                                        # zero the sk>sq region post-exp on the
                                        # idle GpSimd engine: local column j of
                                        # the slice is global sq0+off+j, so
                                        # keep where j - i >= 0.
                                        nc.gpsimd.affine_select(
                                            out=pt_sb[:, 0:w],
                                            in_=pt_sb[:, 0:w],
                                            compare_op=mybir.AluOpType.is_ge,
                                            fill=0.0,
                                            base=0,
                                            pattern=[[1, w]],
                                            channel_multiplier=-1,
                                        )
                                    # PV per 128-chunk so each chunk's psum
                                    # group closes at its true last writer.
                                    # start=True only on the FIRST write of the
                                    # bank occupancy: a start marks the whole
                                    # 2KB zero region pending, so each later
                                    # chunk's first (start=False) write still
                                    # overwrites-and-arms its own bytes, while
                                    # repeated starts would wipe accumulation
                                    # state of already-written chunks.
                                    for c in range(off // 128, 4):
                                        if mode == "causal":
                                            c_stop = kind == "pat" and (ki - 4 * qj) == c
                                        else:
                                            c_stop = n == nlast
                                        nc.tensor.matmul(
                                            ctx_ps[:, c * 128 : (c + 1) * 128],
                                            v_sb[:, ki, h * HD : (h + 1) * HD],
                                            pt_sb[:, c * 128 - off : c * 128 - off + 128],
                                            start=(n == 0 and c == 0),
                                            stop=c_stop,
                                            skip_group_check=True,
                                        )
                                    # per-chunk column sums: pt chunk stationary,
                                    # single moving column -> N=1, nearly free.
                                    for c in range(off // 128, 4):
                                        if mode == "causal":
                                            c_stop = kind == "pat" and (ki - 4 * qj) == c
                                        else:
                                            c_stop = n == nlast
                                        nc.tensor.matmul(
                                            sums_ps[:, c : c + 1],
                                            pt_sb[:, c * 128 - off : c * 128 - off + 128],
                                            onesp1_sb,
                                            start=(n == 0 and c == 0),
                                            stop=c_stop,
                                            skip_group_check=True,
                                        )
                                # normalize: rc = 1/sums in [sq-chunk, 4] layout;
                                # transpose to [4, 128] rows and broadcast across
                                # the hd partitions via 4 rank-1 bf16 matmuls.
                                # Dense do-chunks of the previous quarter are
                                # threaded between the tiny normalize matmuls so
                                # the PE has independent work while the DVE side
                                # of the chain (recip, rcT evac) catches up.
                                rcs_sb = work.tile([128, 4], FD32, tag="rcs")
                                nc.vector.reciprocal(rcs_sb, sums_ps)
                                dsb = None
                                if prev_dense is not None:
                                    dsb = outsb.tile([128, D], BF16, name="dsb")
                                    emit_dense_do(
                                        prev_dense[0],
                                        prev_dense[1],
                                        h,
                                        0,
                                        dps,
                                        dsb,
                                        dve_only=False,
                                    )
                                rcT_ps = smps.tile([4, 128], FD32, tag="smps")
                                nc.tensor.transpose(rcT_ps, rcs_sb, ident_sb)
                                rcT_sb = work.tile([4, 128], BF16, tag="rcT")
                                nc.vector.tensor_copy(rcT_sb, rcT_ps)
                                if prev_dense is not None:
                                    for do in range(1, 4):
                                        emit_dense_do(
                                            prev_dense[0],
                                            prev_dense[1],
                                            h,
                                            do,
                                            dps,
                                            dsb,
                                            dve_only=False,
                                        )
                                bc_ps = scps.tile([128, SQT], FD32, tag="scps")
                                for c in range(4):
                                    # one-hot stationary row c: broadcasts
                                    # rcT row c across all 128 partitions
                                    nc.tensor.matmul(
                                        bc_ps[:, c * 128 : (c + 1) * 128],
                                        oneh_sb[:, c * 128 : (c + 1) * 128],
                                        rcT_sb,
                                        start=(c == 0),
                                        stop=(c == 3),
                                        skip_group_check=True,
                                    )
                                rc_sb = work.tile([128, SQT], BF16, tag="rc")
                                nc.vector.tensor_copy(rc_sb, bc_ps)
                                nc.vector.tensor_tensor(
                                    out=ctxt_sb[:, h, :],
                                    in0=ctx_ps,
                                    in1=rc_sb,
                                    op=mybir.AluOpType.mult,
                                )
                                if prev_dense is not None:
                                    r0 = prev_dense[0] + h * 128
                                    nc.sync.dma_start(
                                        out=outp[r0 : r0 + 128, :], in_=dsb
                                    )
                            prev_dense = (sq0, ctxt_sb)

                    # tail: dense for the last block with full psum freedom
                    with tc.tile_pool(
                        name="dps2", bufs=4, space="PSUM"
                    ) as dps2:
                        emit_dense(
                            prev_dense[0], prev_dense[1], dps2, stream_store=True
                        )

    _split_multi_waits(nc)
    return nc


_PROGRAM_CACHE = {}


def _get_program(mode):
    if mode not in _PROGRAM_CACHE:
        _PROGRAM_CACHE[mode] = _build_program(mode)
    return _PROGRAM_CACHE[mode]


def _classify_mask(mask):
    """mask: [B, 1, S, S] float32 -> 'none' | 'causal' | 'data'."""
    if not np.any(mask):
        return "none"
    tril = np.tril(np.ones((S, S), dtype=bool))
    for b in range(mask.shape[0]):
        m = mask[b, 0]
        if not (np.all(m[tril] == 0.0) and np.all(m[~tril] <= -1.0e8)):
            return "data"
    return "causal"


def kernel(
    hidden_states,
    residual,
    alibi,
    attention_mask,
    W_qkv,
    b_qkv,
    W_dense,
    b_dense,
):
    hidden_states = np.asarray(hidden_states, dtype=np.float32)
    residual = np.asarray(residual, dtype=np.float32)
    alibi = np.asarray(alibi, dtype=np.float32)
    attention_mask = np.asarray(attention_mask, dtype=np.float32)
    W_qkv = np.asarray(W_qkv, dtype=np.float32)
    b_qkv = np.asarray(b_qkv, dtype=np.float32)
    W_dense = np.asarray(W_dense, dtype=np.float32)
    b_dense = np.asarray(b_dense, dtype=np.float32)

    mode = _classify_mask(attention_mask)
    nc = _get_program(mode)

    # W_qkv row blocks per head: rows h*384+[0:128) = q, +128 k, +256 v
    wq = W_qkv.reshape(H, 3, HD, D)[:, 0]  # [H, HD, D]
    wk = W_qkv.reshape(H, 3, HD, D)[:, 1]
    wv = W_qkv.reshape(H, 3, HD, D)[:, 2]
    bq = b_qkv.reshape(H, 3, HD)[:, 0]  # [H, HD]
    bk = b_qkv.reshape(H, 3, HD)[:, 1]
    bv = b_qkv.reshape(H, 3, HD)[:, 2]

    ones1p = np.ones((1, 128), dtype=BF16_NP)
    onesp1 = np.ones((128, 1), dtype=BF16_NP)
    ident_np = np.eye(128, dtype=np.float32)
    oneh_np = np.zeros((4, 4 * 128), dtype=BF16_NP)
    for _c in range(4):
        oneh_np[_c, _c * 128 : (_c + 1) * 128] = 1

    patt_np = None
    if mode == "causal":
        # patt[i, p*512 + j] = -1e9 where (i + 128*p) > j  (sk > sq)
        i_idx = np.arange(128)[:, None]
        j_idx = np.arange(SQT)[None, :]
        blocks = [
            np.where(i_idx + 128 * p > j_idx, np.float32(NEG_BIG), np.float32(0.0))
            for p in range(4)
        ]
        patt_np = np.concatenate(blocks, axis=1).astype(np.float32)

    xt_by_batch = [
        np.ascontiguousarray(hidden_states[b].T).astype(BF16_NP) for b in range(B)
    ]
    maskt_by_batch = None
    if mode == "data":
        # Clamp very-negative mask values: anything <= -190 already gives an
        # exact 0 after exp (given |alibi + qk - c| < ~100), and bounding |c|
        # keeps the bf16 shift vector accurate.
        attention_mask = np.maximum(attention_mask, np.float32(-200.0))
        maskt_by_batch = [
            np.ascontiguousarray(attention_mask[b, 0].T).astype(np.float32)
            for b in range(B)
        ]

    in_maps = []
    for c in range(NCORES):
        b = c // 4
        g = c % 4
        # stride-4 assignment: slot i holds global head g + 4i, so slots 2,3
        # only ever see small-slope heads (8..15) on every core -- required
        # for the shift-free constant-bias path in causal mode.
        heads = [g + 4 * i for i in range(HPC)]

        wq_c = wq[heads].reshape(HPC * HD, D) * INV_NORM  # [512, D]
        wk_c = wk[heads].reshape(HPC * HD, D)
        wv_c = wv[heads].reshape(HPC * HD, D)
        wd_c = W_dense[:, [h * HD + i for h in heads for i in range(HD)]]  # [D, 512]

        bqk_np = np.stack(
            [bq[h] * INV_NORM for h in heads] + [bk[h] for h in heads], axis=1
        ).astype(np.float32)  # [128, 8]
        bvbc_np = np.ascontiguousarray(
            np.broadcast_to(bv[heads].reshape(1, HPC * HD), (128, HPC * HD))
        ).astype(np.float32)

        # per-(head, quarter) alibi bias columns [128, HPC*NQT*NKT] + shift c
        al = np.empty((128, HPC * NQT * NKT), dtype=np.float32)
        al2 = np.empty((128, HPC * NQT * NKT), dtype=np.float32)
        negc_np = np.empty((HPC, S), dtype=np.float32)
        for hl, h in enumerate(heads):
            a = alibi[b * H + h, 0]  # [S]
            if mode == "none":
                c_vec = np.full(S, a.max(), dtype=np.float32)
            elif mode == "causal":
                c_vec = np.maximum.accumulate(a)
            else:
                # c[sq] = max_sk(alibi[sk] + mask[sq, sk])
                c_vec = (a[None, :] + attention_mask[b, 0]).max(axis=1)
            negc_np[hl] = -c_vec
            bias_cols = a.reshape(NKT, 128).T  # [128, NKT]
            for qj in range(4):
                col0 = (hl * 4 + qj) * NKT
                if mode == "none":
                    al[:, col0 : col0 + NKT] = bias_cols - c_vec[0]
                    al2[:, col0 : col0 + NKT] = bias_cols - c_vec[0]
                elif mode == "causal" and hl >= 2:
                    # shift-free slot: fold the per-quarter constant shift
                    al[:, col0 : col0 + NKT] = bias_cols - c_vec[qj * SQT + SQT - 1]
                    al2[:, col0 : col0 + NKT] = al[:, col0 : col0 + NKT]
                elif mode == "causal" and hl == 1:
                    # half-quarter constant shifts: alib covers the first 256
                    # columns of the quarter, alib2 the second 256.
                    al[:, col0 : col0 + NKT] = bias_cols - c_vec[qj * SQT + 255]
                    al2[:, col0 : col0 + NKT] = bias_cols - c_vec[qj * SQT + SQT - 1]
                else:
                    al[:, col0 : col0 + NKT] = bias_cols
                    al2[:, col0 : col0 + NKT] = bias_cols

        im = {
            "xt": xt_by_batch[b],
            "wqt": np.ascontiguousarray(wq_c.T).astype(BF16_NP),
            "wkt": np.ascontiguousarray(wk_c.T).astype(BF16_NP),
            "wvt": np.ascontiguousarray(wv_c.T).astype(BF16_NP),
            "wdt": np.ascontiguousarray(wd_c.T).astype(BF16_NP),
            "bqk": bqk_np,
            "bvbc": bvbc_np,
            "alib": al,
            "alib2": al2,
            "ones1p": ones1p,
            "onesp1": onesp1,
            "ident": ident_np,
            "oneh": oneh_np,
        }
        if mode != "none":
            im["negc"] = negc_np.reshape(1, HPC * S).astype(BF16_NP)
        if mode == "causal":
            im["patt"] = patt_np
        if mode == "data":
            im["maskt"] = maskt_by_batch[b]
        in_maps.append(im)

    res = None
    last_exc = None
    for attempt in range(3):
        try:
            res = bass_utils.run_bass_kernel_spmd(
                nc, in_maps, core_ids=list(range(NCORES))
            )
            break
        except Exception as e:  # transient device wedges (NRT_EXEC_*) happen
            last_exc = e
            time.sleep(2.0 * (attempt + 1))
    if res is None:
        raise last_exc

    out = np.empty((B, S, D), dtype=np.float32)
    for b in range(B):
        acc = b_dense[None, :] + residual[b]
        for g in range(4):
            acc = acc + res.results[b * 4 + g]["outp"].astype(np.float32)
        out[b] = acc
    return out



# revision 62
# speedup vs baseline: 1.0765x; 1.0765x over previous
"""BLOOM attention block (B=2, S=2048, D=2048, H=16) on 8 Trainium2 NeuronCores.

Sharding: core c handles batch b=c//4 and head slot group g=c%4 with a
STRIDE-4 head assignment (slot i holds global head g+4i), so slots 2,3 only
ever see small-ALiBi-slope heads. Each core computes its 4 heads' attention
plus the partial dense projection (W_dense columns for its heads); the host
sums the 4 bf16 partials per batch and adds b_dense + residual in fp32.

Device-side layout avoids all large on-chip transposes:
  - The projection emits Q^T, K^T in [head_dim(=128 partitions), seq] layout
    and V in native [seq, head_dim] layout; the V projection bias is folded
    into V itself during psum evacuation (exact: probs sum to 1).
  - scores are computed transposed: S^T[sk, sq] = K @ Q^T.
  - softmax over sk (the partition dim) uses an analytic shift c[sq] =
    cummax(alibi) per column (any per-column shift cancels in the
    normalization). Slot 0 (big slopes) injects -c via a rank-1 K=1 matmul;
    slot 1 folds per-HALF-quarter constant shifts into two exp biases
    (alib/alib2); slots 2,3 fold a per-quarter constant into one exp bias.
    Underflow analysis: a constant shift over a span W is safe iff
    slope*W + |qk| stays below bf16's exp underflow (~87).
  - column sums come nearly free: per 128-column chunk, a matmul with the
    pt chunk STATIONARY and a ones column moving (N=1, cost-model charges
    by output free size). The sums land as per-partition columns [sq,1],
    are inverted on DVE ([128,4]), PE-transposed to [4,128], and broadcast
    across hd partitions with 4 rank-1 one-hot bf16 matmuls.
  - PSUM zero-region rule: start=True marks the whole 2KB bank pending-zero,
    so only the FIRST write of each bank occupancy carries start=True; each
    later chunk's first write overwrites-and-arms its own bytes.
  - ctx^T[hd, sq] = V^T @ P^T accumulates per 128-chunk in PSUM with
    per-chunk stop at each chunk's true last writer (causal boundary tiles).
  - dense partial OUT[sq, dout] = ctx^T.T @ W_dense^T accumulated over heads;
    dense do-chunks of the previous quarter are interleaved between the
    normalize matmuls of the current quarter's heads to keep the PE fed, and
    the tail quarter streams its stores per do-chunk.
"""

import math
import time

import numpy as np

import bass_rust
import concourse.bass as bass
import concourse.mybir as mybir
import concourse.tile as tile
from concourse import bass_utils

import ml_dtypes

BF16_NP = ml_dtypes.bfloat16

B, S, D, H = 2, 2048, 2048, 16
HD = D // H  # 128
INV_NORM = 1.0 / math.sqrt(HD)
NCORES = 8
HPC = 4  # heads per core
SQT = 512  # sq tile width (free dim of transposed score tiles)
NQT = S // SQT  # 4
NKT = S // 128  # 16 sk tiles
NDT = D // 128  # 16 contraction tiles
FD32 = mybir.dt.float32
BF16 = mybir.dt.bfloat16
NEG_BIG = -1.0e9
PSUM_QPS = 1
PSUM_QKV = 4
WORK_BUFS = 4
PSUM_SMPS = 1
PSUM_SCPS = 3
PSUM_CTXPS = 2
PSUM_DPS = 1
QJ_ORDER = [3, 2, 0, 1]
BOUNDARY_ON_POOL = True
SHARE_QD = False
EXP_SPLIT = False
QX2_BUFS = 2
PT_BUFS = 4
CTXT_BUFS = 2
OUTSB_BUFS = 3


def _split_multi_waits(nc):
    """This toolchain's walrus accepts at most ONE sync wait per instruction;
    Tile emits multi-wait instructions. Move extra waits onto preceding NOPs
    on the same engine (waits execute in stream order, so semantics hold)."""
    for fn in nc.m.functions:
        for bb in fn.blocks:
            insts = bb.instructions
            i = 0
            while i < len(insts):
                inst = insts[i]
                si = inst.sync_info
                if si is not None and len(si.on_wait) > 1:
                    waits = list(si.on_wait)
                    carriers = []
                    for k, w in enumerate(waits[:-1]):
                        nop = mybir.InstNoOp(name=f"{inst.name}_sw{k}", ins=[], outs=[])
                        nop.engine = inst.engine
                        nop.sync_info = bass_rust.SyncInfo(on_wait=[w], on_update=[])
                        nc.register_instruction(nop, overwrite=True)
                        carriers.append(nop)
                    inst.sync_info = bass_rust.SyncInfo(
                        on_wait=[waits[-1]], on_update=si.on_update
                    )
                    insts[i:i] = carriers
                    i += len(carriers)
                i += 1


def _tile_plan(mode):
    """plan[qj][ki] in {'skip','clean','pat'} ('pat' only in causal mode;
    'data' mode returns 'data' everywhere)."""
    plan = []
    for qj in range(NQT):
        row = []
        for ki in range(NKT):
            if mode == "none":
                row.append("clean")
            elif mode == "data":
                row.append("data")
            else:  # causal: keys sk <= queries sq
                sk_lo, sk_hi = 128 * ki, 128 * ki + 127
                sq_lo, sq_hi = SQT * qj, SQT * qj + SQT - 1
                if sk_lo > sq_hi:
                    row.append("skip")
                elif sk_hi <= sq_lo:
                    row.append("clean")
                else:
                    row.append("pat")  # pattern index = ki - 4*qj
        plan.append(row)
    return plan


def _build_program(mode):
    """mode in {'none', 'causal', 'data'}; returns the Bass module."""
    plan = _tile_plan(mode)
    use_shift = mode != "none"  # 'none' folds the constant shift into alib
    # In causal mode slots 2,3 hold only small-slope heads (stride-4 head
    # assignment): a per-(head, quarter) constant shift folded into the exp
    # bias stays within bf16 range, so the rank-1 shift matmul is dropped.
    # Slot 1 (slopes <= 0.177) gets the same treatment with per-HALF-quarter
    # constants (two exp activations per tile, biases from alib/alib2).
    shift_slots = (0,) if mode == "causal" else tuple(range(HPC))
    half_slots = (1,) if mode == "causal" else ()

    nc = bass.Bass()
    xt = nc.dram_tensor("xt", [D, S], BF16, kind="ExternalInput")
    wqt = nc.dram_tensor("wqt", [D, HPC * HD], BF16, kind="ExternalInput")
    wkt = nc.dram_tensor("wkt", [D, HPC * HD], BF16, kind="ExternalInput")
    wvt = nc.dram_tensor("wvt", [D, HPC * HD], BF16, kind="ExternalInput")
    wdt = nc.dram_tensor("wdt", [HPC * HD, D], BF16, kind="ExternalInput")
    bqk = nc.dram_tensor("bqk", [128, 2 * HPC], FD32, kind="ExternalInput")
    bvbc = nc.dram_tensor("bvbc", [128, HPC * HD], FD32, kind="ExternalInput")
    # exp bias per (slot-head, quarter, ki): for shift-free slots the
    # per-quarter constant shift is folded in, so columns vary with qj.
    alib = nc.dram_tensor("alib", [128, HPC * NQT * NKT], FD32, kind="ExternalInput")
    alib2 = nc.dram_tensor("alib2", [128, HPC * NQT * NKT], FD32, kind="ExternalInput")
    ones1p = nc.dram_tensor("ones1p", [1, 128], BF16, kind="ExternalInput")
    onesp1 = nc.dram_tensor("onesp1", [128, 1], BF16, kind="ExternalInput")
    ident = nc.dram_tensor("ident", [128, 128], FD32, kind="ExternalInput")
    oneh = nc.dram_tensor("oneh", [4, 4 * 128], BF16, kind="ExternalInput")
    negc = patt = maskt = None
    if use_shift:
        negc = nc.dram_tensor("negc", [1, HPC * S], BF16, kind="ExternalInput")
    if mode == "causal":
        patt = nc.dram_tensor("patt", [128, 4 * SQT], FD32, kind="ExternalInput")
    if mode == "data":
        maskt = nc.dram_tensor("maskt", [S, S], FD32, kind="ExternalInput")
    outp = nc.dram_tensor("outp", [S, D], BF16, kind="ExternalOutput")

    with tile.TileContext(nc) as tc:
        with tc.tile_pool(name="persist", bufs=1) as persist:
            # ---- persistent SBUF tensors -------------------------------
            # Small constants first (cheap DMAs, needed early).
            qt_sb = persist.tile([128, HPC, S], BF16)  # Q^T per head
            kt_sb = persist.tile([128, HPC, S], BF16)  # K^T per head
            v_sb = persist.tile([128, NKT, HPC * HD], BF16)  # V native
            wdt_sb = persist.tile([128, HPC, D], BF16)
            bqk_sb = persist.tile([128, 2 * HPC], FD32)
            bvbc_sb = persist.tile([128, HPC * HD], FD32)
            alib_sb = persist.tile([128, HPC * NQT * NKT], FD32)
            alib2_sb = persist.tile([128, HPC * NQT * NKT], FD32)
            ones1p_sb = persist.tile([1, 128], BF16)
            onesp1_sb = persist.tile([128, 1], BF16)
            ident_sb = persist.tile([128, 128], FD32)
            oneh_sb = persist.tile([4, 4 * 128], BF16)
            negc_sb = patt_sb = None
            if use_shift:
                negc_sb = persist.tile([1, HPC * S], BF16)
            if mode == "causal":
                patt_sb = persist.tile([128, 4, SQT], FD32)

            def emit_const_dmas():
                # emitted AFTER the critical first weight/xt chunks so the
                # shared descriptor-gen engine serves those first.
                nc.gpsimd.dma_start(out=bqk_sb, in_=bqk[:])
                nc.gpsimd.dma_start(out=bvbc_sb, in_=bvbc[:])
                nc.gpsimd.dma_start(out=alib_sb, in_=alib[:])
                nc.gpsimd.dma_start(out=alib2_sb, in_=alib2[:])
                nc.gpsimd.dma_start(out=ones1p_sb, in_=ones1p[:])
                nc.gpsimd.dma_start(out=onesp1_sb, in_=onesp1[:])
                nc.gpsimd.dma_start(out=ident_sb, in_=ident[:])
                nc.gpsimd.dma_start(out=oneh_sb, in_=oneh[:])
                if use_shift:
                    nc.gpsimd.dma_start(out=negc_sb, in_=negc[:])

            # ---- phase 1: K+V projection (Q is interleaved into phase 2)
            xt_r = xt.rearrange("(dt p) s -> p dt s", p=128)
            wqt_r = wqt.rearrange("(dt p) f -> p dt f", p=128)
            wkt_r = wkt.rearrange("(dt p) f -> p dt f", p=128)
            wvt_r = wvt.rearrange("(dt p) f -> p dt f", p=128)
            with tc.tile_pool(name="wqp", bufs=1) as wqp:
                wq_sb = wqp.tile([128, NDT, HPC * HD], BF16)
                with (
                    tc.tile_pool(name="qkvw", bufs=1) as qkvw,
                    tc.tile_pool(name="qkvx", bufs=2) as qkvx,
                    tc.tile_pool(name="qkvps", bufs=PSUM_QKV, space="PSUM") as qkvps,
                ):
                    # Chunked loads (4 dt-groups each) so the first matmuls
                    # can start as soon as the first chunk lands. The first
                    # wk chunk and first xt chunk go ahead of the constants.
                    wk_sb = qkvw.tile([128, NDT, HPC * HD], BF16)
                    wv_sb = qkvw.tile([128, NDT, HPC * HD], BF16)
                    nc.sync.dma_start(out=wk_sb[:, 0:1, :], in_=wkt_r[:, 0:1, :])
                    xt_q0 = qkvx.tile([128, NDT, SQT], BF16, name="xt_q")
                    nc.scalar.dma_start(out=xt_q0[:, 0:1, :], in_=xt_r[:, 0:1, 0:SQT])
                    nc.sync.dma_start(out=wk_sb[:, 1:4, :], in_=wkt_r[:, 1:4, :])
                    nc.scalar.dma_start(out=xt_q0[:, 1:4, :], in_=xt_r[:, 1:4, 0:SQT])
                    emit_const_dmas()
                    for c4 in range(1, 4):
                        dsl = slice(c4 * 4, (c4 + 1) * 4)
                        nc.sync.dma_start(out=wk_sb[:, dsl, :], in_=wkt_r[:, dsl, :])
                    for c4 in range(4):
                        dsl = slice(c4 * 4, (c4 + 1) * 4)
                        nc.sync.dma_start(out=wv_sb[:, dsl, :], in_=wvt_r[:, dsl, :])
                    for c4 in range(4):
                        dsl = slice(c4 * 4, (c4 + 1) * 4)
                        nc.sync.dma_start(out=wq_sb[:, dsl, :], in_=wqt_r[:, dsl, :])
                    for q in range(4):  # seq quarters of 512
                        sq0 = q * SQT
                        if q == 0:
                            xt_q = xt_q0
                            c4range = range(1, 4)
                        else:
                            xt_q = qkvx.tile([128, NDT, SQT], BF16, name="xt_q")
                            c4range = range(4)
                        for c4 in c4range:
                            dsl = slice(c4 * 4, (c4 + 1) * 4)
                            nc.scalar.dma_start(
                                out=xt_q[:, dsl, :], in_=xt_r[:, dsl, sq0 : sq0 + SQT]
                            )
                        if q == 3:
                            # dense weights are needed only at the first dense
                            # block (~150us in); keep them behind all xt loads.
                            for c4 in range(4):
                                nc.scalar.dma_start(
                                    out=wdt_sb[:, c4, :],
                                    in_=wdt.rearrange("(h p) o -> p h o", p=128)[
                                        :, c4, :
                                    ],
                                )
                        for h in range(HPC):
                            ps_k = qkvps.tile([128, SQT], FD32, tag="qkvps")
                            for dt in range(NDT):
                                nc.tensor.matmul(
                                    ps_k,
                                    wk_sb[:, dt, h * HD : (h + 1) * HD],
                                    xt_q[:, dt, :],
                                    start=(dt == 0),
                                    stop=(dt == NDT - 1),
                                )
                            nc.vector.tensor_scalar_add(
                                kt_sb[:, h, sq0 : sq0 + SQT],
                                ps_k,
                                bqk_sb[:, HPC + h : HPC + h + 1],
                            )
                        for sc in range(4):  # V rows within the quarter
                            ps_v = qkvps.tile([128, SQT], FD32, tag="qkvps")
                            for dt in range(NDT):
                                nc.tensor.matmul(
                                    ps_v,
                                    xt_q[:, dt, sc * 128 : (sc + 1) * 128],
                                    wv_sb[:, dt, :],
                                    start=(dt == 0),
                                    stop=(dt == NDT - 1),
                                )
                            # V carries its projection bias: exact, since the
                            # normalized probs per column sum to 1, so ctx/sum
                            # picks up + bv without a separate rank-1 fold.
                            nc.vector.tensor_tensor(
                                out=v_sb[:, q * 4 + sc, :],
                                in0=ps_v,
                                in1=bvbc_sb,
                                op=mybir.AluOpType.add,
                            )
                        if q == QJ_ORDER[0]:
                            # Q for the first attention block: computed here
                            # while its xt quarter is still resident, so
                            # attention can start the moment K/V complete.
                            for h in range(HPC):
                                ps_q = qkvps.tile([128, SQT], FD32, tag="qkvps")
                                for dt in range(NDT):
                                    nc.tensor.matmul(
                                        ps_q,
                                        wq_sb[:, dt, h * HD : (h + 1) * HD],
                                        xt_q[:, dt, :],
                                        start=(dt == 0),
                                        stop=(dt == NDT - 1),
                                    )
                                nc.vector.tensor_scalar_add(
                                    qt_sb[:, h, sq0 : sq0 + SQT],
                                    ps_q,
                                    bqk_sb[:, h : h + 1],
                                )

                # ---- phases 2+3: Q projection + attention + dense, per sq
                # block of 512; Q matmuls interleave with attention to keep
                # the PE fed across unit boundaries.
                with (
                    tc.tile_pool(name="qx2", bufs=QX2_BUFS) as qx2,
                    tc.tile_pool(name="work", bufs=WORK_BUFS) as work,
                    tc.tile_pool(name="ctxtp", bufs=CTXT_BUFS) as ctxtp,
                    tc.tile_pool(name="outsb", bufs=OUTSB_BUFS) as outsb,
                    tc.tile_pool(name="maskp", bufs=2) as maskp,
                ):

                    def emit_dense_do(
                        sq0, ctxt_sb, sc, do, pool, out_sb, tag="dps", dve_only=False
                    ):
                        o_ps = pool.tile([128, 512], FD32, tag=tag, name="o_ps")
                        for h in range(HPC):
                            nc.tensor.matmul(
                                o_ps,
                                ctxt_sb[:, h, sc * 128 : (sc + 1) * 128],
                                wdt_sb[:, h, do * 512 : (do + 1) * 512],
                                start=(h == 0),
                                stop=(h == HPC - 1),
                            )
                        if dve_only or do % 2 == 0:
                            nc.vector.tensor_copy(
                                out_sb[:, do * 512 : (do + 1) * 512], o_ps
                            )
                        else:
                            nc.scalar.copy(out_sb[:, do * 512 : (do + 1) * 512], o_ps)

                    def emit_dense(sq0, ctxt_sb, pool, tag="dps", stream_store=False):
                        for sc in range(4):
                            r0 = sq0 + sc * 128
                            if stream_store:
                                # tail: stream each do-chunk's store right
                                # after its evac copy.
                                out_sb = outsb.tile([128, D], BF16, name="out_sb")
                                for do in range(4):
                                    emit_dense_do(
                                        sq0, ctxt_sb, sc, do, pool, out_sb, tag
                                    )
                                    nc.sync.dma_start(
                                        out=outp[
                                            r0 : r0 + 128, do * 512 : (do + 1) * 512
                                        ],
                                        in_=out_sb[:, do * 512 : (do + 1) * 512],
                                    )
                            else:
                                out_sb = outsb.tile([128, D], BF16, name="out_sb")
                                for do in range(4):
                                    emit_dense_do(
                                        sq0, ctxt_sb, sc, do, pool, out_sb, tag
                                    )
                                nc.sync.dma_start(
                                    out=outp[r0 : r0 + 128, :], in_=out_sb
                                )

                    prev_dense = None
                    with (
                        tc.tile_pool(name="qps", bufs=max(PSUM_QPS, 1), space="PSUM") as qps0,
                        tc.tile_pool(
                            name="scps", bufs=PSUM_SCPS, space="PSUM"
                        ) as scps,
                        tc.tile_pool(
                            name="ctxps", bufs=PSUM_CTXPS, space="PSUM"
                        ) as ctxps,
                        tc.tile_pool(name="smps", bufs=PSUM_SMPS, space="PSUM") as smps,
                        tc.tile_pool(name="dps", bufs=PSUM_DPS, space="PSUM") as dps,
                    ):
                        qps = qps0 if PSUM_QPS > 0 else scps
                        qtag = "qps" if PSUM_QPS > 0 else "scps"
                        for qj in QJ_ORDER:
                            sq0 = qj * SQT
                            if qj != QJ_ORDER[0]:
                                xt_q = qx2.tile([128, NDT, SQT], BF16)
                                for c4 in range(4):
                                    dsl = slice(c4 * 4, (c4 + 1) * 4)
                                    nc.scalar.dma_start(
                                        out=xt_q[:, dsl, :],
                                        in_=xt_r[:, dsl, sq0 : sq0 + SQT],
                                    )
                                for h in range(HPC):
                                    ps_q = qps.tile([128, SQT], FD32, tag=qtag, name="ps_q")
                                    for dt in range(NDT):
                                        nc.tensor.matmul(
                                            ps_q,
                                            wq_sb[:, dt, h * HD : (h + 1) * HD],
                                            xt_q[:, dt, :],
                                            start=(dt == 0),
                                            stop=(dt == NDT - 1),
                                        )
                                    nc.vector.tensor_scalar_add(
                                        qt_sb[:, h, sq0 : sq0 + SQT],
                                        ps_q,
                                        bqk_sb[:, h : h + 1],
                                    )
                            ctxt_sb = ctxtp.tile([128, HPC, SQT], BF16)
                            for h in range(HPC):
                                ki_list = [
                                    ki for ki in range(NKT) if plan[qj][ki] != "skip"
                                ]
                                nlast = len(ki_list) - 1
                                ctx_ps = ctxps.tile([128, SQT], FD32, tag="ctxps")
                                sums_ps = smps.tile([128, 4], FD32, tag="smps")
                                for n, ki in enumerate(ki_list):
                                    kind = plan[qj][ki]
                                    # boundary tiles: sq columns below the
                                    # diagonal block are fully masked -- skip
                                    # them (the first tile of each unit is
                                    # always full width, so the psum
                                    # accumulation start covers all columns).
                                    off = 0
                                    if kind == "pat":
                                        off = 128 * (ki - 4 * qj)
                                    w = SQT - off
                                    q0o = sq0 + off
                                    h_shift = use_shift and h in shift_slots
                                    s_ps = scps.tile([128, SQT], FD32, tag="scps")
                                    if h_shift:
                                        nc.tensor.matmul(
                                            s_ps[:, off:SQT],
                                            ones1p_sb,
                                            negc_sb[0:1, h * S + q0o : h * S + sq0 + SQT],
                                            start=True,
                                            stop=False,
                                        )
                                    nc.tensor.matmul(
                                        s_ps[:, off:SQT],
                                        kt_sb[:, h, ki * 128 : (ki + 1) * 128],
                                        qt_sb[:, h, q0o : sq0 + SQT],
                                        start=not h_shift,
                                        stop=True,
                                    )
                                    if kind == "pat" and not BOUNDARY_ON_POOL:
                                        nc.vector.tensor_tensor(
                                            out=s_ps[:, off:SQT],
                                            in0=s_ps[:, off:SQT],
                                            in1=patt_sb[:, ki - 4 * qj, off:SQT],
                                            op=mybir.AluOpType.add,
                                        )
                                    elif kind == "data":
                                        mk_sb = maskp.tile([128, SQT], FD32, tag="mask")
                                        nc.sync.dma_start(
                                            out=mk_sb,
                                            in_=maskt[
                                                ki * 128 : (ki + 1) * 128, sq0 : sq0 + SQT
                                            ],
                                        )
                                        nc.vector.tensor_tensor(
                                            out=s_ps,
                                            in0=s_ps,
                                            in1=mk_sb,
                                            op=mybir.AluOpType.add,
                                        )
                                    pt_sb = work.tile([128, SQT], BF16, tag="pt", bufs=PT_BUFS)
                                    bcol = (h * NQT + qj) * NKT + ki
                                    if h in half_slots:
                                        # per-half-quarter constant shifts:
                                        # first half bias from alib, second
                                        # from alib2 (both fold their own c).
                                        if off < 256:
                                            nc.scalar.activation(
                                                pt_sb[:, 0 : 256 - off],
                                                s_ps[:, off:256],
                                                mybir.ActivationFunctionType.Exp,
                                                bias=alib_sb[:, bcol : bcol + 1],
                                            )
                                        lo = max(off, 256)
                                        nc.scalar.activation(
                                            pt_sb[:, lo - off : SQT - off],
                                            s_ps[:, lo:SQT],
                                            mybir.ActivationFunctionType.Exp,
                                            bias=alib2_sb[:, bcol : bcol + 1],
                                        )
                                    else:
                                        nc.scalar.activation(
                                            pt_sb[:, 0:w],
                                            s_ps[:, off:SQT],
                                            mybir.ActivationFunctionType.Exp,
                                            bias=alib_sb[:, bcol : bcol + 1],
                                        )
                                    if kind == "pat" and BOUNDARY_ON_POOL:
                                        # zero the sk>sq region post-exp on the
                                        # idle GpSimd engine: local column j of
                                        # the slice is global sq0+off+j, so
                                        # keep where j - i >= 0.
                                        nc.gpsimd.affine_select(
                                            out=pt_sb[:, 0:w],
                                            in_=pt_sb[:, 0:w],
                                            compare_op=mybir.AluOpType.is_ge,
                                            fill=0.0,
                                            base=0,
                                            pattern=[[1, w]],
                                            channel_multiplier=-1,
                                        )
                                    # PV per 128-chunk so each chunk's psum
                                    # group closes at its true last writer.
                                    # start=True only on the FIRST write of the
                                    # bank occupancy: a start marks the whole
                                    # 2KB zero region pending, so each later
                                    # chunk's first (start=False) write still
                                    # overwrites-and-arms its own bytes, while
                                    # repeated starts would wipe accumulation
                                    # state of already-written chunks.
                                    for c in range(off // 128, 4):
                                        if mode == "causal":
                                            c_stop = kind == "pat" and (ki - 4 * qj) == c
                                        else:
                                            c_stop = n == nlast
                                        nc.tensor.matmul(
                                            ctx_ps[:, c * 128 : (c + 1) * 128],
                                            v_sb[:, ki, h * HD : (h + 1) * HD],
                                            pt_sb[:, c * 128 - off : c * 128 - off + 128],
                                            start=(n == 0 and c == 0),
                                            stop=c_stop,
                                            skip_group_check=True,
                                        )
                                    # per-chunk column sums: pt chunk stationary,
                                    # single moving column -> N=1, nearly free.
                                    for c in range(off // 128, 4):
                                        if mode == "causal":
                                            c_stop = kind == "pat" and (ki - 4 * qj) == c
                                        else:
                                            c_stop = n == nlast
                                        nc.tensor.matmul(
                                            sums_ps[:, c : c + 1],
                                            pt_sb[:, c * 128 - off : c * 128 - off + 128],
                                            onesp1_sb,
                                            start=(n == 0 and c == 0),
                                            stop=c_stop,
                                            skip_group_check=True,
                                        )
                                # normalize: rc = 1/sums in [sq-chunk, 4] layout;
                                # transpose to [4, 128] rows and broadcast across
                                # the hd partitions via 4 rank-1 bf16 matmuls.
                                # Dense do-chunks of the previous quarter are
                                # threaded between the tiny normalize matmuls so
                                # the PE has independent work while the DVE side
                                # of the chain (recip, rcT evac) catches up.
                                rcs_sb = work.tile([128, 4], FD32, tag="rcs")
                                nc.vector.reciprocal(rcs_sb, sums_ps)
                                dsb = None
                                if prev_dense is not None:
                                    dsb = outsb.tile([128, D], BF16, name="dsb")
                                    emit_dense_do(
                                        prev_dense[0],
                                        prev_dense[1],
                                        h,
                                        0,
                                        dps,
                                        dsb,
                                        dve_only=False,
                                    )
                                rcT_ps = smps.tile([4, 128], FD32, tag="smps")
                                nc.tensor.transpose(rcT_ps, rcs_sb, ident_sb)
                                rcT_sb = work.tile([4, 128], BF16, tag="rcT")
                                nc.vector.tensor_copy(rcT_sb, rcT_ps)
                                if prev_dense is not None:
                                    for do in range(1, 4):
                                        emit_dense_do(
                                            prev_dense[0],
                                            prev_dense[1],
                                            h,
                                            do,
                                            dps,
                                            dsb,
                                            dve_only=False,
                                        )
                                bc_ps = scps.tile([128, SQT], FD32, tag="scps")
                                for c in range(4):
                                    # one-hot stationary row c: broadcasts
                                    # rcT row c across all 128 partitions
                                    nc.tensor.matmul(
                                        bc_ps[:, c * 128 : (c + 1) * 128],
                                        oneh_sb[:, c * 128 : (c + 1) * 128],
                                        rcT_sb,
                                        start=(c == 0),
                                        stop=(c == 3),
                                        skip_group_check=True,
                                    )
                                rc_sb = work.tile([128, SQT], BF16, tag="rc")
                                nc.scalar.copy(rc_sb, bc_ps)
                                nc.vector.tensor_tensor(
                                    out=ctxt_sb[:, h, :],
                                    in0=ctx_ps,
                                    in1=rc_sb,
                                    op=mybir.AluOpType.mult,
                                )
                                if prev_dense is not None:
                                    r0 = prev_dense[0] + h * 128
                                    nc.sync.dma_start(
                                        out=outp[r0 : r0 + 128, :], in_=dsb
                                    )
                            prev_dense = (sq0, ctxt_sb)

                    # tail: dense for the last block with full psum freedom
                    with tc.tile_pool(
                        name="dps2", bufs=4, space="PSUM"
                    ) as dps2:
                        emit_dense(
                            prev_dense[0], prev_dense[1], dps2, stream_store=True
                        )

    _split_multi_waits(nc)
    return nc


_PROGRAM_CACHE = {}


def _get_program(mode):
    if mode not in _PROGRAM_CACHE:
        _PROGRAM_CACHE[mode] = _build_program(mode)
    return _PROGRAM_CACHE[mode]


def _classify_mask(mask):
    """mask: [B, 1, S, S] float32 -> 'none' | 'causal' | 'data'."""
    if not np.any(mask):
        return "none"
    tril = np.tril(np.ones((S, S), dtype=bool))
    for b in range(mask.shape[0]):
        m = mask[b, 0]
        if not (np.all(m[tril] == 0.0) and np.all(m[~tril] <= -1.0e8)):
            return "data"
    return "causal"


def kernel(
    hidden_states,
    residual,
    alibi,
    attention_mask,
    W_qkv,
    b_qkv,
    W_dense,
    b_dense,
):
    hidden_states = np.asarray(hidden_states, dtype=np.float32)
    residual = np.asarray(residual, dtype=np.float32)
    alibi = np.asarray(alibi, dtype=np.float32)
    attention_mask = np.asarray(attention_mask, dtype=np.float32)
    W_qkv = np.asarray(W_qkv, dtype=np.float32)
    b_qkv = np.asarray(b_qkv, dtype=np.float32)
    W_dense = np.asarray(W_dense, dtype=np.float32)
    b_dense = np.asarray(b_dense, dtype=np.float32)

    mode = _classify_mask(attention_mask)
    nc = _get_program(mode)

    # W_qkv row blocks per head: rows h*384+[0:128) = q, +128 k, +256 v
    wq = W_qkv.reshape(H, 3, HD, D)[:, 0]  # [H, HD, D]
    wk = W_qkv.reshape(H, 3, HD, D)[:, 1]
    wv = W_qkv.reshape(H, 3, HD, D)[:, 2]
    bq = b_qkv.reshape(H, 3, HD)[:, 0]  # [H, HD]
    bk = b_qkv.reshape(H, 3, HD)[:, 1]
    bv = b_qkv.reshape(H, 3, HD)[:, 2]

    ones1p = np.ones((1, 128), dtype=BF16_NP)
    onesp1 = np.ones((128, 1), dtype=BF16_NP)
    ident_np = np.eye(128, dtype=np.float32)
    oneh_np = np.zeros((4, 4 * 128), dtype=BF16_NP)
    for _c in range(4):
        oneh_np[_c, _c * 128 : (_c + 1) * 128] = 1

    patt_np = None
    if mode == "causal":
        # patt[i, p*512 + j] = -1e9 where (i + 128*p) > j  (sk > sq)
        i_idx = np.arange(128)[:, None]
        j_idx = np.arange(SQT)[None, :]
        blocks = [
            np.where(i_idx + 128 * p > j_idx, np.float32(NEG_BIG), np.float32(0.0))
            for p in range(4)
        ]
        patt_np = np.concatenate(blocks, axis=1).astype(np.float32)

    xt_by_batch = [
        np.ascontiguousarray(hidden_states[b].T).astype(BF16_NP) for b in range(B)
    ]
    maskt_by_batch = None
    if mode == "data":
        # Clamp very-negative mask values: anything <= -190 already gives an
        # exact 0 after exp (given |alibi + qk - c| < ~100), and bounding |c|
        # keeps the bf16 shift vector accurate.
        attention_mask = np.maximum(attention_mask, np.float32(-200.0))
        maskt_by_batch = [
            np.ascontiguousarray(attention_mask[b, 0].T).astype(np.float32)
            for b in range(B)
        ]

    in_maps = []
    for c in range(NCORES):
        b = c // 4
        g = c % 4
        # stride-4 assignment: slot i holds global head g + 4i, so slots 2,3
        # only ever see small-slope heads (8..15) on every core -- required
        # for the shift-free constant-bias path in causal mode.
        heads = [g + 4 * i for i in range(HPC)]

        wq_c = wq[heads].reshape(HPC * HD, D) * INV_NORM  # [512, D]
        wk_c = wk[heads].reshape(HPC * HD, D)
        wv_c = wv[heads].reshape(HPC * HD, D)
        wd_c = W_dense[:, [h * HD + i for h in heads for i in range(HD)]]  # [D, 512]

        bqk_np = np.stack(
            [bq[h] * INV_NORM for h in heads] + [bk[h] for h in heads], axis=1
        ).astype(np.float32)  # [128, 8]
        bvbc_np = np.ascontiguousarray(
            np.broadcast_to(bv[heads].reshape(1, HPC * HD), (128, HPC * HD))
        ).astype(np.float32)

        # per-(head, quarter) alibi bias columns [128, HPC*NQT*NKT] + shift c
        al = np.empty((128, HPC * NQT * NKT), dtype=np.float32)
        al2 = np.empty((128, HPC * NQT * NKT), dtype=np.float32)
        negc_np = np.empty((HPC, S), dtype=np.float32)
        for hl, h in enumerate(heads):
            a = alibi[b * H + h, 0]  # [S]
            if mode == "none":
                c_vec = np.full(S, a.max(), dtype=np.float32)
            elif mode == "causal":
                c_vec = np.maximum.accumulate(a)
            else:
                # c[sq] = max_sk(alibi[sk] + mask[sq, sk])
                c_vec = (a[None, :] + attention_mask[b, 0]).max(axis=1)
            negc_np[hl] = -c_vec
            bias_cols = a.reshape(NKT, 128).T  # [128, NKT]
            for qj in range(4):
                col0 = (hl * 4 + qj) * NKT
                if mode == "none":
                    al[:, col0 : col0 + NKT] = bias_cols - c_vec[0]
                    al2[:, col0 : col0 + NKT] = bias_cols - c_vec[0]
                elif mode == "causal" and hl >= 2:
                    # shift-free slot: fold the per-quarter constant shift
                    al[:, col0 : col0 + NKT] = bias_cols - c_vec[qj * SQT + SQT - 1]
                    al2[:, col0 : col0 + NKT] = al[:, col0 : col0 + NKT]
                elif mode == "causal" and hl == 1:
                    # half-quarter constant shifts: alib covers the first 256
                    # columns of the quarter, alib2 the second 256.
                    al[:, col0 : col0 + NKT] = bias_cols - c_vec[qj * SQT + 255]
                    al2[:, col0 : col0 + NKT] = bias_cols - c_vec[qj * SQT + SQT - 1]
                else:
                    al[:, col0 : col0 + NKT] = bias_cols
                    al2[:, col0 : col0 + NKT] = bias_cols

        im = {
            "xt": xt_by_batch[b],
            "wqt": np.ascontiguousarray(wq_c.T).astype(BF16_NP),
            "wkt": np.ascontiguousarray(wk_c.T).astype(BF16_NP),
            "wvt": np.ascontiguousarray(wv_c.T).astype(BF16_NP),
            "wdt": np.ascontiguousarray(wd_c.T).astype(BF16_NP),
            "bqk": bqk_np,
            "bvbc": bvbc_np,
            "alib": al,
            "alib2": al2,
            "ones1p": ones1p,
            "onesp1": onesp1,
            "ident": ident_np,
            "oneh": oneh_np,
        }
        if mode != "none":
            im["negc"] = negc_np.reshape(1, HPC * S).astype(BF16_NP)
        if mode == "causal":
            im["patt"] = patt_np
        if mode == "data":
            im["maskt"] = maskt_by_batch[b]
        in_maps.append(im)

    res = None
    last_exc = None
    for attempt in range(3):
        try:
            res = bass_utils.run_bass_kernel_spmd(
                nc, in_maps, core_ids=list(range(NCORES))
            )
            break
        except Exception as e:  # transient device wedges (NRT_EXEC_*) happen
            last_exc = e
            time.sleep(2.0 * (attempt + 1))
    if res is None:
        raise last_exc

    out = np.empty((B, S, D), dtype=np.float32)
    for b in range(B):
        acc = b_dense[None, :] + residual[b]
        for g in range(4):
            acc = acc + res.results[b * 4 + g]["outp"].astype(np.float32)
        out[b] = acc
    return out



# revision 65
# speedup vs baseline: 1.0939x; 1.0162x over previous
"""BLOOM attention block (B=2, S=2048, D=2048, H=16) on 8 Trainium2 NeuronCores.

Sharding: core c handles batch b=c//4 and head slot group g=c%4 with a
STRIDE-4 head assignment (slot i holds global head g+4i), so slots 2,3 only
ever see small-ALiBi-slope heads. Each core computes its 4 heads' attention
plus the partial dense projection (W_dense columns for its heads); the host
sums the 4 bf16 partials per batch and adds b_dense + residual in fp32.

Device-side layout avoids all large on-chip transposes:
  - The projection emits Q^T, K^T in [head_dim(=128 partitions), seq] layout
    and V in native [seq, head_dim] layout; the V projection bias is folded
    into V itself during psum evacuation (exact: probs sum to 1).
  - scores are computed transposed: S^T[sk, sq] = K @ Q^T.
  - softmax over sk (the partition dim) uses an analytic shift c[sq] =
    cummax(alibi) per column (any per-column shift cancels in the
    normalization). Slot 0 (big slopes) injects -c via a rank-1 K=1 matmul;
    slot 1 folds per-HALF-quarter constant shifts into two exp biases
    (alib/alib2); slots 2,3 fold a per-quarter constant into one exp bias.
    Underflow analysis: a constant shift over a span W is safe iff
    slope*W + |qk| stays below bf16's exp underflow (~87).
  - column sums come nearly free: per 128-column chunk, a matmul with the
    pt chunk STATIONARY and a ones column moving (N=1, cost-model charges
    by output free size). The sums land as per-partition columns [sq,1],
    are inverted on DVE ([128,4]), PE-transposed to [4,128], and broadcast
    across hd partitions with 4 rank-1 one-hot bf16 matmuls.
  - PSUM zero-region rule: start=True marks the whole 2KB bank pending-zero,
    so only the FIRST write of each bank occupancy carries start=True; each
    later chunk's first write overwrites-and-arms its own bytes.
  - ctx^T[hd, sq] = V^T @ P^T accumulates per 128-chunk in PSUM with
    per-chunk stop at each chunk's true last writer (causal boundary tiles).
  - dense partial OUT[sq, dout] = ctx^T.T @ W_dense^T accumulated over heads;
    dense do-chunks of the previous quarter are interleaved between the
    normalize matmuls of the current quarter's heads to keep the PE fed, and
    the tail quarter streams its stores per do-chunk.
"""

import math
import time

import numpy as np

import bass_rust
import concourse.bass as bass
import concourse.mybir as mybir
import concourse.tile as tile
from concourse import bass_utils

import ml_dtypes

BF16_NP = ml_dtypes.bfloat16

B, S, D, H = 2, 2048, 2048, 16
HD = D // H  # 128
INV_NORM = 1.0 / math.sqrt(HD)
NCORES = 8
HPC = 4  # heads per core
SQT = 512  # sq tile width (free dim of transposed score tiles)
NQT = S // SQT  # 4
NKT = S // 128  # 16 sk tiles
NDT = D // 128  # 16 contraction tiles
FD32 = mybir.dt.float32
BF16 = mybir.dt.bfloat16
NEG_BIG = -1.0e9
PSUM_QPS = 1
PSUM_QKV = 4
WORK_BUFS = 4
PSUM_SMPS = 1
PSUM_SCPS = 3
PSUM_CTXPS = 2
PSUM_DPS = 1
QJ_ORDER = [3, 2, 0, 1]
BOUNDARY_ON_POOL = True
SHARE_QD = False
EXP_SPLIT = False
QX2_BUFS = 2
PT_BUFS = 4
CTXT_BUFS = 2
OUTSB_BUFS = 3


def _split_multi_waits(nc):
    """This toolchain's walrus accepts at most ONE sync wait per instruction;
    Tile emits multi-wait instructions. Move extra waits onto preceding NOPs
    on the same engine (waits execute in stream order, so semantics hold)."""
    for fn in nc.m.functions:
        for bb in fn.blocks:
            insts = bb.instructions
            i = 0
            while i < len(insts):
                inst = insts[i]
                si = inst.sync_info
                if si is not None and len(si.on_wait) > 1:
                    waits = list(si.on_wait)
                    carriers = []
                    for k, w in enumerate(waits[:-1]):
                        nop = mybir.InstNoOp(name=f"{inst.name}_sw{k}", ins=[], outs=[])
                        nop.engine = inst.engine
                        nop.sync_info = bass_rust.SyncInfo(on_wait=[w], on_update=[])
                        nc.register_instruction(nop, overwrite=True)
                        carriers.append(nop)
                    inst.sync_info = bass_rust.SyncInfo(
                        on_wait=[waits[-1]], on_update=si.on_update
                    )
                    insts[i:i] = carriers
                    i += len(carriers)
                i += 1


def _tile_plan(mode):
    """plan[qj][ki] in {'skip','clean','pat'} ('pat' only in causal mode;
    'data' mode returns 'data' everywhere)."""
    plan = []
    for qj in range(NQT):
        row = []
        for ki in range(NKT):
            if mode == "none":
                row.append("clean")
            elif mode == "data":
                row.append("data")
            else:  # causal: keys sk <= queries sq
                sk_lo, sk_hi = 128 * ki, 128 * ki + 127
                sq_lo, sq_hi = SQT * qj, SQT * qj + SQT - 1
                if sk_lo > sq_hi:
                    row.append("skip")
                elif sk_hi <= sq_lo:
                    row.append("clean")
                else:
                    row.append("pat")  # pattern index = ki - 4*qj
        plan.append(row)
    return plan


def _build_program(mode):
    """mode in {'none', 'causal', 'data'}; returns the Bass module."""
    plan = _tile_plan(mode)
    use_shift = mode != "none"  # 'none' folds the constant shift into alib
    # In causal mode slots 2,3 hold only small-slope heads (stride-4 head
    # assignment): a per-(head, quarter) constant shift folded into the exp
    # bias stays within bf16 range, so the rank-1 shift matmul is dropped.
    # Slot 1 (slopes <= 0.177) gets the same treatment with per-HALF-quarter
    # constants (two exp activations per tile, biases from alib/alib2).
    shift_slots = (0,) if mode == "causal" else tuple(range(HPC))
    half_slots = (1,) if mode == "causal" else ()

    nc = bass.Bass()
    xt = nc.dram_tensor("xt", [D, S], BF16, kind="ExternalInput")
    wqt = nc.dram_tensor("wqt", [D, HPC * HD], BF16, kind="ExternalInput")
    wkt = nc.dram_tensor("wkt", [D, HPC * HD], BF16, kind="ExternalInput")
    wvt = nc.dram_tensor("wvt", [D, HPC * HD], BF16, kind="ExternalInput")
    wdt = nc.dram_tensor("wdt", [HPC * HD, D], BF16, kind="ExternalInput")
    bqk = nc.dram_tensor("bqk", [128, 2 * HPC], FD32, kind="ExternalInput")
    bvbc = nc.dram_tensor("bvbc", [128, HPC * HD], FD32, kind="ExternalInput")
    # exp bias per (slot-head, quarter, ki): for shift-free slots the
    # per-quarter constant shift is folded in, so columns vary with qj.
    alib = nc.dram_tensor("alib", [128, HPC * NQT * NKT], FD32, kind="ExternalInput")
    alib2 = nc.dram_tensor("alib2", [128, HPC * NQT * NKT], FD32, kind="ExternalInput")
    ones1p = nc.dram_tensor("ones1p", [1, 128], BF16, kind="ExternalInput")
    onesp1 = nc.dram_tensor("onesp1", [128, 1], BF16, kind="ExternalInput")
    ident = nc.dram_tensor("ident", [128, 128], FD32, kind="ExternalInput")
    mask01 = nc.dram_tensor("mask01", [128, SQT], BF16, kind="ExternalInput")
    oneh = nc.dram_tensor("oneh", [4, 4 * 128], BF16, kind="ExternalInput")
    negc = patt = maskt = None
    if use_shift:
        negc = nc.dram_tensor("negc", [1, HPC * S], BF16, kind="ExternalInput")
    if mode == "causal":
        patt = nc.dram_tensor("patt", [128, 4 * SQT], FD32, kind="ExternalInput")
    if mode == "data":
        maskt = nc.dram_tensor("maskt", [S, S], FD32, kind="ExternalInput")
    outp = nc.dram_tensor("outp", [S, D], BF16, kind="ExternalOutput")

    with tile.TileContext(nc) as tc:
        with tc.tile_pool(name="persist", bufs=1) as persist:
            # ---- persistent SBUF tensors -------------------------------
            # Small constants first (cheap DMAs, needed early).
            qt_sb = persist.tile([128, HPC, S], BF16)  # Q^T per head
            kt_sb = persist.tile([128, HPC, S], BF16)  # K^T per head
            v_sb = persist.tile([128, NKT, HPC * HD], BF16)  # V native
            wdt_sb = persist.tile([128, HPC, D], BF16)
            bqk_sb = persist.tile([128, 2 * HPC], FD32)
            bvbc_sb = persist.tile([128, HPC * HD], FD32)
            alib_sb = persist.tile([128, HPC * NQT * NKT], FD32)
            alib2_sb = persist.tile([128, HPC * NQT * NKT], FD32)
            ones1p_sb = persist.tile([1, 128], BF16)
            onesp1_sb = persist.tile([128, 1], BF16)
            ident_sb = persist.tile([128, 128], FD32)
            mask01_sb = persist.tile([128, SQT], BF16)
            oneh_sb = persist.tile([4, 4 * 128], BF16)
            negc_sb = patt_sb = None
            if use_shift:
                negc_sb = persist.tile([1, HPC * S], BF16)
            if mode == "causal":
                patt_sb = persist.tile([128, 4, SQT], FD32)

            def emit_const_dmas():
                # emitted AFTER the critical first weight/xt chunks so the
                # shared descriptor-gen engine serves those first.
                nc.gpsimd.dma_start(out=bqk_sb, in_=bqk[:])
                nc.gpsimd.dma_start(out=bvbc_sb, in_=bvbc[:])
                nc.gpsimd.dma_start(out=alib_sb, in_=alib[:])
                nc.gpsimd.dma_start(out=alib2_sb, in_=alib2[:])
                nc.gpsimd.dma_start(out=ones1p_sb, in_=ones1p[:])
                nc.gpsimd.dma_start(out=onesp1_sb, in_=onesp1[:])
                nc.gpsimd.dma_start(out=ident_sb, in_=ident[:])
                nc.gpsimd.dma_start(out=mask01_sb, in_=mask01[:])
                nc.gpsimd.dma_start(out=oneh_sb, in_=oneh[:])
                if use_shift:
                    nc.gpsimd.dma_start(out=negc_sb, in_=negc[:])

            # ---- phase 1: K+V projection (Q is interleaved into phase 2)
            xt_r = xt.rearrange("(dt p) s -> p dt s", p=128)
            wqt_r = wqt.rearrange("(dt p) f -> p dt f", p=128)
            wkt_r = wkt.rearrange("(dt p) f -> p dt f", p=128)
            wvt_r = wvt.rearrange("(dt p) f -> p dt f", p=128)
            with tc.tile_pool(name="wqp", bufs=1) as wqp:
                wq_sb = wqp.tile([128, NDT, HPC * HD], BF16)
                with (
                    tc.tile_pool(name="qkvw", bufs=1) as qkvw,
                    tc.tile_pool(name="qkvx", bufs=2) as qkvx,
                    tc.tile_pool(name="qkvps", bufs=PSUM_QKV, space="PSUM") as qkvps,
                ):
                    # Chunked loads (4 dt-groups each) so the first matmuls
                    # can start as soon as the first chunk lands. The first
                    # wk chunk and first xt chunk go ahead of the constants.
                    wk_sb = qkvw.tile([128, NDT, HPC * HD], BF16)
                    wv_sb = qkvw.tile([128, NDT, HPC * HD], BF16)
                    nc.sync.dma_start(out=wk_sb[:, 0:1, :], in_=wkt_r[:, 0:1, :])
                    xt_q0 = qkvx.tile([128, NDT, SQT], BF16, name="xt_q")
                    nc.scalar.dma_start(out=xt_q0[:, 0:1, :], in_=xt_r[:, 0:1, 0:SQT])
                    nc.sync.dma_start(out=wk_sb[:, 1:4, :], in_=wkt_r[:, 1:4, :])
                    nc.scalar.dma_start(out=xt_q0[:, 1:4, :], in_=xt_r[:, 1:4, 0:SQT])
                    emit_const_dmas()
                    for c4 in range(1, 4):
                        dsl = slice(c4 * 4, (c4 + 1) * 4)
                        nc.sync.dma_start(out=wk_sb[:, dsl, :], in_=wkt_r[:, dsl, :])
                    for c4 in range(4):
                        dsl = slice(c4 * 4, (c4 + 1) * 4)
                        nc.sync.dma_start(out=wv_sb[:, dsl, :], in_=wvt_r[:, dsl, :])
                    for c4 in range(4):
                        dsl = slice(c4 * 4, (c4 + 1) * 4)
                        nc.sync.dma_start(out=wq_sb[:, dsl, :], in_=wqt_r[:, dsl, :])
                    for q in range(4):  # seq quarters of 512
                        sq0 = q * SQT
                        if q == 0:
                            xt_q = xt_q0
                            c4range = range(1, 4)
                        else:
                            xt_q = qkvx.tile([128, NDT, SQT], BF16, name="xt_q")
                            c4range = range(4)
                        for c4 in c4range:
                            dsl = slice(c4 * 4, (c4 + 1) * 4)
                            nc.scalar.dma_start(
                                out=xt_q[:, dsl, :], in_=xt_r[:, dsl, sq0 : sq0 + SQT]
                            )
                        if q == 3:
                            # dense weights are needed only at the first dense
                            # block (~150us in); keep them behind all xt loads.
                            for c4 in range(4):
                                nc.scalar.dma_start(
                                    out=wdt_sb[:, c4, :],
                                    in_=wdt.rearrange("(h p) o -> p h o", p=128)[
                                        :, c4, :
                                    ],
                                )
                        for h in range(HPC):
                            ps_k = qkvps.tile([128, SQT], FD32, tag="qkvps")
                            for dt in range(NDT):
                                nc.tensor.matmul(
                                    ps_k,
                                    wk_sb[:, dt, h * HD : (h + 1) * HD],
                                    xt_q[:, dt, :],
                                    start=(dt == 0),
                                    stop=(dt == NDT - 1),
                                )
                            nc.vector.tensor_scalar_add(
                                kt_sb[:, h, sq0 : sq0 + SQT],
                                ps_k,
                                bqk_sb[:, HPC + h : HPC + h + 1],
                            )
                        for sc in range(4):  # V rows within the quarter
                            ps_v = qkvps.tile([128, SQT], FD32, tag="qkvps")
                            for dt in range(NDT):
                                nc.tensor.matmul(
                                    ps_v,
                                    xt_q[:, dt, sc * 128 : (sc + 1) * 128],
                                    wv_sb[:, dt, :],
                                    start=(dt == 0),
                                    stop=(dt == NDT - 1),
                                )
                            # V carries its projection bias: exact, since the
                            # normalized probs per column sum to 1, so ctx/sum
                            # picks up + bv without a separate rank-1 fold.
                            nc.vector.tensor_tensor(
                                out=v_sb[:, q * 4 + sc, :],
                                in0=ps_v,
                                in1=bvbc_sb,
                                op=mybir.AluOpType.add,
                            )
                        if q == QJ_ORDER[0]:
                            # Q for the first attention block: computed here
                            # while its xt quarter is still resident, so
                            # attention can start the moment K/V complete.
                            for h in range(HPC):
                                ps_q = qkvps.tile([128, SQT], FD32, tag="qkvps")
                                for dt in range(NDT):
                                    nc.tensor.matmul(
                                        ps_q,
                                        wq_sb[:, dt, h * HD : (h + 1) * HD],
                                        xt_q[:, dt, :],
                                        start=(dt == 0),
                                        stop=(dt == NDT - 1),
                                    )
                                nc.vector.tensor_scalar_add(
                                    qt_sb[:, h, sq0 : sq0 + SQT],
                                    ps_q,
                                    bqk_sb[:, h : h + 1],
                                )

                # ---- phases 2+3: Q projection + attention + dense, per sq
                # block of 512; Q matmuls interleave with attention to keep
                # the PE fed across unit boundaries.
                with (
                    tc.tile_pool(name="qx2", bufs=QX2_BUFS) as qx2,
                    tc.tile_pool(name="work", bufs=WORK_BUFS) as work,
                    tc.tile_pool(name="ctxtp", bufs=CTXT_BUFS) as ctxtp,
                    tc.tile_pool(name="outsb", bufs=OUTSB_BUFS) as outsb,
                    tc.tile_pool(name="maskp", bufs=2) as maskp,
                ):

                    def emit_dense_do(
                        sq0, ctxt_sb, sc, do, pool, out_sb, tag="dps", dve_only=False
                    ):
                        o_ps = pool.tile([128, 512], FD32, tag=tag, name="o_ps")
                        for h in range(HPC):
                            nc.tensor.matmul(
                                o_ps,
                                ctxt_sb[:, h, sc * 128 : (sc + 1) * 128],
                                wdt_sb[:, h, do * 512 : (do + 1) * 512],
                                start=(h == 0),
                                stop=(h == HPC - 1),
                            )
                        if dve_only or do % 2 == 0:
                            nc.vector.tensor_copy(
                                out_sb[:, do * 512 : (do + 1) * 512], o_ps
                            )
                        else:
                            nc.scalar.copy(out_sb[:, do * 512 : (do + 1) * 512], o_ps)

                    def emit_dense(sq0, ctxt_sb, pool, tag="dps", stream_store=False):
                        for sc in range(4):
                            r0 = sq0 + sc * 128
                            if stream_store:
                                # tail: stream each do-chunk's store right
                                # after its evac copy.
                                out_sb = outsb.tile([128, D], BF16, name="out_sb")
                                for do in range(4):
                                    emit_dense_do(
                                        sq0, ctxt_sb, sc, do, pool, out_sb, tag
                                    )
                                    nc.sync.dma_start(
                                        out=outp[
                                            r0 : r0 + 128, do * 512 : (do + 1) * 512
                                        ],
                                        in_=out_sb[:, do * 512 : (do + 1) * 512],
                                    )
                            else:
                                out_sb = outsb.tile([128, D], BF16, name="out_sb")
                                for do in range(4):
                                    emit_dense_do(
                                        sq0, ctxt_sb, sc, do, pool, out_sb, tag
                                    )
                                nc.sync.dma_start(
                                    out=outp[r0 : r0 + 128, :], in_=out_sb
                                )

                    prev_dense = None
                    with (
                        tc.tile_pool(name="qps", bufs=max(PSUM_QPS, 1), space="PSUM") as qps0,
                        tc.tile_pool(
                            name="scps", bufs=PSUM_SCPS, space="PSUM"
                        ) as scps,
                        tc.tile_pool(
                            name="ctxps", bufs=PSUM_CTXPS, space="PSUM"
                        ) as ctxps,
                        tc.tile_pool(name="smps", bufs=PSUM_SMPS, space="PSUM") as smps,
                        tc.tile_pool(name="dps", bufs=PSUM_DPS, space="PSUM") as dps,
                    ):
                        qps = qps0 if PSUM_QPS > 0 else scps
                        qtag = "qps" if PSUM_QPS > 0 else "scps"
                        for qj in QJ_ORDER:
                            sq0 = qj * SQT
                            if qj != QJ_ORDER[0]:
                                xt_q = qx2.tile([128, NDT, SQT], BF16)
                                for c4 in range(4):
                                    dsl = slice(c4 * 4, (c4 + 1) * 4)
                                    nc.scalar.dma_start(
                                        out=xt_q[:, dsl, :],
                                        in_=xt_r[:, dsl, sq0 : sq0 + SQT],
                                    )
                                for h in range(HPC):
                                    ps_q = qps.tile([128, SQT], FD32, tag=qtag, name="ps_q")
                                    for dt in range(NDT):
                                        nc.tensor.matmul(
                                            ps_q,
                                            wq_sb[:, dt, h * HD : (h + 1) * HD],
                                            xt_q[:, dt, :],
                                            start=(dt == 0),
                                            stop=(dt == NDT - 1),
                                        )
                                    nc.vector.tensor_scalar_add(
                                        qt_sb[:, h, sq0 : sq0 + SQT],
                                        ps_q,
                                        bqk_sb[:, h : h + 1],
                                    )
                            ctxt_sb = ctxtp.tile([128, HPC, SQT], BF16)
                            for h in range(HPC):
                                ki_list = [
                                    ki for ki in range(NKT) if plan[qj][ki] != "skip"
                                ]
                                nlast = len(ki_list) - 1
                                ctx_ps = ctxps.tile([128, SQT], FD32, tag="ctxps")
                                sums_ps = smps.tile([128, 4], FD32, tag="smps")
                                for n, ki in enumerate(ki_list):
                                    kind = plan[qj][ki]
                                    # boundary tiles: sq columns below the
                                    # diagonal block are fully masked -- skip
                                    # them (the first tile of each unit is
                                    # always full width, so the psum
                                    # accumulation start covers all columns).
                                    off = 0
                                    if kind == "pat":
                                        off = 128 * (ki - 4 * qj)
                                    w = SQT - off
                                    q0o = sq0 + off
                                    h_shift = use_shift and h in shift_slots
                                    s_ps = scps.tile([128, SQT], FD32, tag="scps")
                                    if h_shift:
                                        nc.tensor.matmul(
                                            s_ps[:, off:SQT],
                                            ones1p_sb,
                                            negc_sb[0:1, h * S + q0o : h * S + sq0 + SQT],
                                            start=True,
                                            stop=False,
                                        )
                                    nc.tensor.matmul(
                                        s_ps[:, off:SQT],
                                        kt_sb[:, h, ki * 128 : (ki + 1) * 128],
                                        qt_sb[:, h, q0o : sq0 + SQT],
                                        start=not h_shift,
                                        stop=True,
                                    )
                                    if kind == "pat" and not BOUNDARY_ON_POOL:
                                        nc.vector.tensor_tensor(
                                            out=s_ps[:, off:SQT],
                                            in0=s_ps[:, off:SQT],
                                            in1=patt_sb[:, ki - 4 * qj, off:SQT],
                                            op=mybir.AluOpType.add,
                                        )
                                    elif kind == "data":
                                        mk_sb = maskp.tile([128, SQT], FD32, tag="mask")
                                        nc.sync.dma_start(
                                            out=mk_sb,
                                            in_=maskt[
                                                ki * 128 : (ki + 1) * 128, sq0 : sq0 + SQT
                                            ],
                                        )
                                        nc.vector.tensor_tensor(
                                            out=s_ps,
                                            in0=s_ps,
                                            in1=mk_sb,
                                            op=mybir.AluOpType.add,
                                        )
                                    pt_sb = work.tile([128, SQT], BF16, tag="pt", bufs=PT_BUFS)
                                    bcol = (h * NQT + qj) * NKT + ki
                                    if h in half_slots:
                                        # per-half-quarter constant shifts:
                                        # first half bias from alib, second
                                        # from alib2 (both fold their own c).
                                        if off < 256:
                                            nc.scalar.activation(
                                                pt_sb[:, 0 : 256 - off],
                                                s_ps[:, off:256],
                                                mybir.ActivationFunctionType.Exp,
                                                bias=alib_sb[:, bcol : bcol + 1],
                                            )
                                        lo = max(off, 256)
                                        nc.scalar.activation(
                                            pt_sb[:, lo - off : SQT - off],
                                            s_ps[:, lo:SQT],
                                            mybir.ActivationFunctionType.Exp,
                                            bias=alib2_sb[:, bcol : bcol + 1],
                                        )
                                    else:
                                        nc.scalar.activation(
                                            pt_sb[:, 0:w],
                                            s_ps[:, off:SQT],
                                            mybir.ActivationFunctionType.Exp,
                                            bias=alib_sb[:, bcol : bcol + 1],
                                        )
                                    if (
                                        kind == "pat"
                                        and BOUNDARY_ON_POOL
                                        and h not in shift_slots
                                    ):
                                        # slots without the rank-1 shift have
                                        # bounded exponents (no inf in pt), so
                                        # the boundary zeroing can be a cheap
                                        # 0/1 mask multiply on the DVE; the
                                        # keep-predicate j >= i is the same
                                        # for every boundary pattern.
                                        nc.vector.tensor_tensor(
                                            out=pt_sb[:, 0:w],
                                            in0=pt_sb[:, 0:w],
                                            in1=mask01_sb[:, 0:w],
                                            op=mybir.AluOpType.mult,
                                        )
                                    elif kind == "pat" and BOUNDARY_ON_POOL:
                                        # slot 0: pt may hold inf (masked
                                        # entries above the shift bound), so
                                        # REPLACE via affine_select: keep
                                        # where j - i >= 0.
                                        nc.gpsimd.affine_select(
                                            out=pt_sb[:, 0:w],
                                            in_=pt_sb[:, 0:w],
                                            compare_op=mybir.AluOpType.is_ge,
                                            fill=0.0,
                                            base=0,
                                            pattern=[[1, w]],
                                            channel_multiplier=-1,
                                        )
                                    # PV per 128-chunk so each chunk's psum
                                    # group closes at its true last writer.
                                    # start=True only on the FIRST write of the
                                    # bank occupancy: a start marks the whole
                                    # 2KB zero region pending, so each later
                                    # chunk's first (start=False) write still
                                    # overwrites-and-arms its own bytes, while
                                    # repeated starts would wipe accumulation
                                    # state of already-written chunks.
                                    for c in range(off // 128, 4):
                                        if mode == "causal":
                                            c_stop = kind == "pat" and (ki - 4 * qj) == c
                                        else:
                                            c_stop = n == nlast
                                        nc.tensor.matmul(
                                            ctx_ps[:, c * 128 : (c + 1) * 128],
                                            v_sb[:, ki, h * HD : (h + 1) * HD],
                                            pt_sb[:, c * 128 - off : c * 128 - off + 128],
                                            start=(n == 0 and c == 0),
                                            stop=c_stop,
                                            skip_group_check=True,
                                        )
                                    # per-chunk column sums: pt chunk stationary,
                                    # single moving column -> N=1, nearly free.
                                    for c in range(off // 128, 4):
                                        if mode == "causal":
                                            c_stop = kind == "pat" and (ki - 4 * qj) == c
                                        else:
                                            c_stop = n == nlast
                                        nc.tensor.matmul(
                                            sums_ps[:, c : c + 1],
                                            pt_sb[:, c * 128 - off : c * 128 - off + 128],
                                            onesp1_sb,
                                            start=(n == 0 and c == 0),
                                            stop=c_stop,
                                            skip_group_check=True,
                                        )
                                # normalize: rc = 1/sums in [sq-chunk, 4] layout;
                                # transpose to [4, 128] rows and broadcast across
                                # the hd partitions via 4 rank-1 bf16 matmuls.
                                # Dense do-chunks of the previous quarter are
                                # threaded between the tiny normalize matmuls so
                                # the PE has independent work while the DVE side
                                # of the chain (recip, rcT evac) catches up.
                                rcs_sb = work.tile([128, 4], FD32, tag="rcs")
                                nc.vector.reciprocal(rcs_sb, sums_ps)
                                dsb = None
                                if prev_dense is not None:
                                    dsb = outsb.tile([128, D], BF16, name="dsb")
                                    emit_dense_do(
                                        prev_dense[0],
                                        prev_dense[1],
                                        h,
                                        0,
                                        dps,
                                        dsb,
                                        dve_only=False,
                                    )
                                rcT_ps = smps.tile([4, 128], FD32, tag="smps")
                                nc.tensor.transpose(rcT_ps, rcs_sb, ident_sb)
                                rcT_sb = work.tile([4, 128], BF16, tag="rcT")
                                nc.vector.tensor_copy(rcT_sb, rcT_ps)
                                if prev_dense is not None:
                                    for do in range(1, 4):
                                        emit_dense_do(
                                            prev_dense[0],
                                            prev_dense[1],
                                            h,
                                            do,
                                            dps,
                                            dsb,
                                            dve_only=False,
                                        )
                                bc_ps = scps.tile([128, SQT], FD32, tag="scps")
                                for c in range(4):
                                    # one-hot stationary row c: broadcasts
                                    # rcT row c across all 128 partitions
                                    nc.tensor.matmul(
                                        bc_ps[:, c * 128 : (c + 1) * 128],
                                        oneh_sb[:, c * 128 : (c + 1) * 128],
                                        rcT_sb,
                                        start=(c == 0),
                                        stop=(c == 3),
                                        skip_group_check=True,
                                    )
                                rc_sb = work.tile([128, SQT], BF16, tag="rc")
                                nc.scalar.copy(rc_sb, bc_ps)
                                nc.vector.tensor_tensor(
                                    out=ctxt_sb[:, h, :],
                                    in0=ctx_ps,
                                    in1=rc_sb,
                                    op=mybir.AluOpType.mult,
                                )
                                if prev_dense is not None:
                                    r0 = prev_dense[0] + h * 128
                                    nc.sync.dma_start(
                                        out=outp[r0 : r0 + 128, :], in_=dsb
                                    )
                            prev_dense = (sq0, ctxt_sb)

                    # tail: dense for the last block with full psum freedom
                    with tc.tile_pool(
                        name="dps2", bufs=4, space="PSUM"
                    ) as dps2:
                        emit_dense(
                            prev_dense[0], prev_dense[1], dps2, stream_store=True
                        )

    _split_multi_waits(nc)
    return nc


_PROGRAM_CACHE = {}


def _get_program(mode):
    if mode not in _PROGRAM_CACHE:
        _PROGRAM_CACHE[mode] = _build_program(mode)
    return _PROGRAM_CACHE[mode]


def _classify_mask(mask):
    """mask: [B, 1, S, S] float32 -> 'none' | 'causal' | 'data'."""
    if not np.any(mask):
        return "none"
    tril = np.tril(np.ones((S, S), dtype=bool))
    for b in range(mask.shape[0]):
        m = mask[b, 0]
        if not (np.all(m[tril] == 0.0) and np.all(m[~tril] <= -1.0e8)):
            return "data"
    return "causal"


def kernel(
    hidden_states,
    residual,
    alibi,
    attention_mask,
    W_qkv,
    b_qkv,
    W_dense,
    b_dense,
):
    hidden_states = np.asarray(hidden_states, dtype=np.float32)
    residual = np.asarray(residual, dtype=np.float32)
    alibi = np.asarray(alibi, dtype=np.float32)
    attention_mask = np.asarray(attention_mask, dtype=np.float32)
    W_qkv = np.asarray(W_qkv, dtype=np.float32)
    b_qkv = np.asarray(b_qkv, dtype=np.float32)
    W_dense = np.asarray(W_dense, dtype=np.float32)
    b_dense = np.asarray(b_dense, dtype=np.float32)

    mode = _classify_mask(attention_mask)
    nc = _get_program(mode)

    # W_qkv row blocks per head: rows h*384+[0:128) = q, +128 k, +256 v
    wq = W_qkv.reshape(H, 3, HD, D)[:, 0]  # [H, HD, D]
    wk = W_qkv.reshape(H, 3, HD, D)[:, 1]
    wv = W_qkv.reshape(H, 3, HD, D)[:, 2]
    bq = b_qkv.reshape(H, 3, HD)[:, 0]  # [H, HD]
    bk = b_qkv.reshape(H, 3, HD)[:, 1]
    bv = b_qkv.reshape(H, 3, HD)[:, 2]

    ones1p = np.ones((1, 128), dtype=BF16_NP)
    onesp1 = np.ones((128, 1), dtype=BF16_NP)
    ident_np = np.eye(128, dtype=np.float32)
    mask01_np = (
        np.arange(SQT)[None, :] >= np.arange(128)[:, None]
    ).astype(BF16_NP)
    oneh_np = np.zeros((4, 4 * 128), dtype=BF16_NP)
    for _c in range(4):
        oneh_np[_c, _c * 128 : (_c + 1) * 128] = 1

    patt_np = None
    if mode == "causal":
        # patt[i, p*512 + j] = -1e9 where (i + 128*p) > j  (sk > sq)
        i_idx = np.arange(128)[:, None]
        j_idx = np.arange(SQT)[None, :]
        blocks = [
            np.where(i_idx + 128 * p > j_idx, np.float32(NEG_BIG), np.float32(0.0))
            for p in range(4)
        ]
        patt_np = np.concatenate(blocks, axis=1).astype(np.float32)

    xt_by_batch = [
        np.ascontiguousarray(hidden_states[b].T).astype(BF16_NP) for b in range(B)
    ]
    maskt_by_batch = None
    if mode == "data":
        # Clamp very-negative mask values: anything <= -190 already gives an
        # exact 0 after exp (given |alibi + qk - c| < ~100), and bounding |c|
        # keeps the bf16 shift vector accurate.
        attention_mask = np.maximum(attention_mask, np.float32(-200.0))
        maskt_by_batch = [
            np.ascontiguousarray(attention_mask[b, 0].T).astype(np.float32)
            for b in range(B)
        ]

    in_maps = []
    for c in range(NCORES):
        b = c // 4
        g = c % 4
        # stride-4 assignment: slot i holds global head g + 4i, so slots 2,3
        # only ever see small-slope heads (8..15) on every core -- required
        # for the shift-free constant-bias path in causal mode.
        heads = [g + 4 * i for i in range(HPC)]

        wq_c = wq[heads].reshape(HPC * HD, D) * INV_NORM  # [512, D]
        wk_c = wk[heads].reshape(HPC * HD, D)
        wv_c = wv[heads].reshape(HPC * HD, D)
        wd_c = W_dense[:, [h * HD + i for h in heads for i in range(HD)]]  # [D, 512]

        bqk_np = np.stack(
            [bq[h] * INV_NORM for h in heads] + [bk[h] for h in heads], axis=1
        ).astype(np.float32)  # [128, 8]
        bvbc_np = np.ascontiguousarray(
            np.broadcast_to(bv[heads].reshape(1, HPC * HD), (128, HPC * HD))
        ).astype(np.float32)

        # per-(head, quarter) alibi bias columns [128, HPC*NQT*NKT] + shift c
        al = np.empty((128, HPC * NQT * NKT), dtype=np.float32)
        al2 = np.empty((128, HPC * NQT * NKT), dtype=np.float32)
        negc_np = np.empty((HPC, S), dtype=np.float32)
        for hl, h in enumerate(heads):
            a = alibi[b * H + h, 0]  # [S]
            if mode == "none":
                c_vec = np.full(S, a.max(), dtype=np.float32)
            elif mode == "causal":
                c_vec = np.maximum.accumulate(a)
            else:
                # c[sq] = max_sk(alibi[sk] + mask[sq, sk])
                c_vec = (a[None, :] + attention_mask[b, 0]).max(axis=1)
            negc_np[hl] = -c_vec
            bias_cols = a.reshape(NKT, 128).T  # [128, NKT]
            for qj in range(4):
                col0 = (hl * 4 + qj) * NKT
                if mode == "none":
                    al[:, col0 : col0 + NKT] = bias_cols - c_vec[0]
                    al2[:, col0 : col0 + NKT] = bias_cols - c_vec[0]
                elif mode == "causal" and hl >= 2:
                    # shift-free slot: fold the per-quarter constant shift
                    al[:, col0 : col0 + NKT] = bias_cols - c_vec[qj * SQT + SQT - 1]
                    al2[:, col0 : col0 + NKT] = al[:, col0 : col0 + NKT]
                elif mode == "causal" and hl == 1:
                    # half-quarter constant shifts: alib covers the first 256
                    # columns of the quarter, alib2 the second 256.
                    al[:, col0 : col0 + NKT] = bias_cols - c_vec[qj * SQT + 255]
                    al2[:, col0 : col0 + NKT] = bias_cols - c_vec[qj * SQT + SQT - 1]
                else:
                    al[:, col0 : col0 + NKT] = bias_cols
                    al2[:, col0 : col0 + NKT] = bias_cols

        im = {
            "xt": xt_by_batch[b],
            "wqt": np.ascontiguousarray(wq_c.T).astype(BF16_NP),
            "wkt": np.ascontiguousarray(wk_c.T).astype(BF16_NP),
            "wvt": np.ascontiguousarray(wv_c.T).astype(BF16_NP),
            "wdt": np.ascontiguousarray(wd_c.T).astype(BF16_NP),
            "bqk": bqk_np,
            "bvbc": bvbc_np,
            "alib": al,
            "alib2": al2,
            "ones1p": ones1p,
            "onesp1": onesp1,
            "ident": ident_np,
            "mask01": mask01_np,
            "oneh": oneh_np,
        }
        if mode != "none":
            im["negc"] = negc_np.reshape(1, HPC * S).astype(BF16_NP)
        if mode == "causal":
            im["patt"] = patt_np
        if mode == "data":
            im["maskt"] = maskt_by_batch[b]
        in_maps.append(im)

    res = None
    last_exc = None
    for attempt in range(3):
        try:
            res = bass_utils.run_bass_kernel_spmd(
                nc, in_maps, core_ids=list(range(NCORES))
            )
            break
        except Exception as e:  # transient device wedges (NRT_EXEC_*) happen
            last_exc = e
            time.sleep(2.0 * (attempt + 1))
    if res is None:
        raise last_exc

    out = np.empty((B, S, D), dtype=np.float32)
    for b in range(B):
        acc = b_dense[None, :] + residual[b]
        for g in range(4):
            acc = acc + res.results[b * 4 + g]["outp"].astype(np.float32)
        out[b] = acc
    return out

